# revision 6
# baseline (speedup 1.0000x reference)
"""Trainium2 Bass kernel for nn_Convolution_1176821039998.

Equivariant (e3nn-style) 3D convolution, kernel 5x5x5, 64->64 channels, on a
[1,64,56,56,56] fp32 volume, plus a per-irrep self-connection on the cropped
volume.  Strategy:

Host side (tiny, fp32):
  - Build the dense conv kernel K[o,i,dz,dy,dx] from the TP weight exactly as
    the reference does, and fold the self-connection into the center tap.
  - Perfectly balanced z-shard across 8 cores with NO redundant compute:
    core c computes 6 "main" output planes 6c..6c+5 (planes 0..47) plus a
    13-line y-block of one of the remaining 2 plane-pairs (planes 48..51,
    block chosen by core index).  The partial block's input sub-volume is
    packed by the host into a small side tensor at FIXED local coordinates,
    so all cores run the identical SPMD program (3.25 plane-pairs each).
  - The kernel halo (4 planes) is handled by overlapping shards; no
    device-to-device exchange.

Zero-tap structure: the equivariant kernel is EXACTLY zero for taps with
lattice distance >= 1.25 (the smooth_finite radial embedding vanishes):
  K[:,:,dz,dy,dx] == 0  iff  (dz-2)^2+(dy-2)^2+(dx-2)^2 >= 7.
Per (dy,dx) column with rho2=(dy-2)^2+(dx-2)^2: rho2<=2 (9 cols) keeps all
5 dz taps; rho2 in {4,5} (12 cols) keeps only dz in {1,2,3}; the 4 corner
columns are fully zero and skipped.

Device side (fp8e4m3 DoubleRow matmuls, 0.5 PE cycles per output column):
  - Operands are split hi/lo: xs=8*x -> xhi=e4m3(xs), xlo=e4m3(xs-xhi);
    Ks=32*K -> Khi, Klo likewise.  The product is computed as
    Khi*xhi + Klo*xhi + Khi*xlo (the Klo*xlo term is dropped; measured
    max-rel-err of the scheme vs fp32 reference is ~2e-3).  Host divides
    the gathered output by 256.
  - SBUF holds dual z-shifted copies (partitions 0..63 plane j, 64..127
    plane j+1) of the hi and lo volumes.  A DoubleRow matmul contracts TWO
    such k-tiles (at a constant plane stride, expressed as a strided slice)
    against a [128, 2, 128] fp8 weight pair, accumulating 4 plane-taps per
    instruction into a PSUM pair of output planes (M = 64 ch x 2 planes).
  - Per (dy,dx) column and plane-pair: rho2<=2 needs k-tile slots
    {0,0,2,2,4,4} (hi, classes hi/lo per plane) + {0,2,4} (lo volume, hi
    class) -> 5 DR matmuls; rho2 in {4,5}: slots {1,1,3,3} + {1,3} -> 3.
    81 DR matmuls per chunk (vs 51 fp16 matmuls = 2x fewer PE cycles).
"""

import os
import numpy as np
import ml_dtypes

import concourse.bass as bass
import concourse.mybir as mybir
import concourse.tile as tile
from concourse import bacc
from concourse.bass_utils import run_bass_kernel_spmd

# ---------------------------------------------------------------- constants
SIZE = 5
MUL = 16
CROP = SIZE // 2
PW0 = np.float32((1.0 / 32.0) ** 0.5)
PW1 = np.float32((3.0 / 32.0) ** 0.5)
INV_SQRT3 = np.float32(3.0 ** -0.5)

N_CORES = 8
S = 56                                 # input spatial size
SO = 52                                # output spatial size
# main shard: 3 pairs = 6 output planes at z0 = 6c (covers 0..47)
N_PAIRS = 3
D_OUT = 6
D_DRAM = 10                            # main DRAM slab planes (6c..6c+9)
D_SB = 9                               # planes per SBUF copy (lo 0..8, hi 1..9)
# partial shard: planes 48..51 split into 8 (pair, 13-line block) quarters
P2_BASE = 48
P2_LINES = 13
P2_IN_LINES = P2_LINES + 4             # 17
P2_DRAM = 6                            # partial DRAM planes (P..P+5)
P2_SB = 5                              # partial SBUF planes per copy
CHUNKS = [(0, 9), (9, 9), (18, 9), (27, 9), (36, 9), (45, 7)]  # (y0, lines)
# last pair ends with a tiny chunk so the final PSUM-evict + store DMA tail
# after the last matmul is short
CHUNKS_LAST = [(0, 9), (9, 9), (18, 9), (27, 9), (36, 9), (45, 5), (50, 2)]
CHUNKS2 = [(0, 7), (7, 6)]             # partial 13-line block

E4 = ml_dtypes.float8_e4m3fn
SX = np.float32(8.0)                   # x scale before fp8 quantization
SK = np.float32(32.0)                  # K scale before fp8 quantization
OUT_SCALE = np.float32(1.0 / (8.0 * 32.0))


def _dr_groups():
    """DoubleRow matmul groups per (chunk x plane-pair).

    Each entry: (dy, dx, kind, slotA, slotB, clsA, clsB).
    kind 0 = hi volume, 1 = lo volume.  slot = dual-copy slot relative to the
    pair base z (slot j holds planes z+j / z+j+1 in its two partition
    halves).  cls 0 = Khi, 1 = Klo, None = zero-weight padding half.
    Ordered in phases so early groups only touch low slots (DMA gating).
    """
    cols = [(dy, dx) for dy in range(5) for dx in range(5)
            if (dy - 2) ** 2 + (dx - 2) ** 2 <= 5]

    def full(dy, dx):
        return (dy - 2) ** 2 + (dx - 2) ** 2 <= 2

    phases = ([], [], [], [], [])
    for dy, dx in cols:
        if full(dy, dx):
            phases[0].append((dy, dx, 0, 0, 2, 0, 0))
            phases[1].append((dy, dx, 0, 2, 4, 1, 0))
            phases[2].append((dy, dx, 0, 0, 4, 1, 1))
            phases[3].append((dy, dx, 1, 0, 2, 0, 0))
            # the odd lo k-tile pairs with a free slot; use it for a partial
            # 4th-term (Klo*xlo, planes 2..3) correction at zero extra cost
            phases[4].append((dy, dx, 1, 2, 4, 1, 0))
        else:
            phases[0].append((dy, dx, 0, 1, 3, 0, 0))
            phases[1].append((dy, dx, 0, 1, 3, 1, 1))
            phases[3].append((dy, dx, 1, 1, 3, 0, 0))
    return [g for ph in phases for g in ph]


GROUPS = _dr_groups()
NG = len(GROUPS)  # 81


def _core_assign(c):
    """(main z0, partial pair base, partial y0) for core c."""
    return 6 * c, P2_BASE + 2 * (c // 4), P2_LINES * (c % 4)


# ------------------------------------------------------- host-side weights
def _lattice_consts():
    r = np.linspace(-1.0, 1.0, SIZE, dtype=np.float32)
    lat = np.stack(np.meshgrid(r, r, r, indexing="ij"), axis=-1)
    d = np.linalg.norm(lat.astype(np.float64), axis=-1).astype(np.float32)
    values = np.linspace(0.0, 1.0, SIZE, dtype=np.float32)
    step = values[1] - values[0]
    diff = (d[..., None] - values) / step

    def sus(t):
        return np.where(t > 0, np.exp(-1.0 / np.where(t > 0, t, 1.0)), 0.0).astype(
            np.float32
        )

    emb = np.float32(1.14136) * np.float32(np.e ** 2) * sus(diff + 1.0) * sus(1.0 - diff)
    n = lat / np.maximum(d, 1e-12)[..., None]
    sh0 = np.ones_like(d)
    sh1 = np.float32(3.0 ** 0.5) * n
    return emb.astype(np.float32), sh0, sh1.astype(np.float32)


def _make_kernel(weight):
    """[5,1024] -> conv kernel [out=64, in=64, 5,5,5] fp32 (mirrors reference)."""
    emb, sh0, sh1 = _lattice_consts()
    w = emb @ weight
    Ssp = w.shape[:3]
    blk = MUL * MUL
    w1, w2, w3, w4 = [
        w[..., i * blk : (i + 1) * blk].reshape(*Ssp, MUL, MUL) for i in range(4)
    ]
    k_ss = PW0 * w1 * sh0[..., None, None]
    k_sv = PW1 * INV_SQRT3 * np.einsum("...uw,...k->...uwk", w2, sh1)
    k_vs = PW0 * INV_SQRT3 * np.einsum("...uw,...i->...uiw", w4, sh1)
    eye3 = np.eye(3, dtype=w.dtype)
    k_vv = (
        PW1
        * INV_SQRT3
        * (w3 * sh0[..., None, None])[..., :, None, :, None]
        * eye3[None, None, None, None, :, None, :]
    )
    top = np.concatenate([k_ss, k_sv.reshape(*Ssp, MUL, 3 * MUL)], axis=-1)
    bot = np.concatenate(
        [k_vs.reshape(*Ssp, 3 * MUL, MUL), k_vv.reshape(*Ssp, 3 * MUL, 3 * MUL)],
        axis=-1,
    )
    kernel = np.concatenate([top, bot], axis=-2)  # [5,5,5,in,out]
    return np.ascontiguousarray(np.transpose(kernel, (4, 3, 0, 1, 2)))


def _fold_self_connection(K, w_sc0, w_sc1):
    """Add the cropped e3nn Linear self-connection into the center tap."""
    inv = np.float32(1.0 / MUL ** 0.5)
    sc = np.zeros((64, 64), np.float32)
    sc[:MUL, :MUL] = w_sc0.T * inv  # sc[out w, in u] = w_sc0[u, w]
    for wo in range(MUL):
        for u in range(MUL):
            for k in range(3):
                sc[MUL + 3 * wo + k, MUL + 3 * u + k] += w_sc1[u, wo] * inv
    K = K.copy()
    K[:, :, CROP, CROP, CROP] += sc
    return K


def _pack_weights(K):
    """[64,64,5,5,5] fp32 -> DoubleRow lhsT tiles [128, NG, 2, 128] fp8.

    Row-half r of k-tile t holds the blocks for plane slot+r; column half m
    (out plane z+m) holds tap dz = plane - m from Khi or Klo per the group's
    class assignment."""
    Ks = K * SK
    Khi = Ks.astype(E4).astype(np.float32)
    Klo = (Ks - Khi).astype(E4).astype(np.float32)
    wk = np.zeros((128, NG, 2, 128), np.float32)
    for g, (dy, dx, kind, sA, sB, cA, cB) in enumerate(GROUPS):
        for t, (slot, cls) in enumerate(((sA, cA), (sB, cB))):
            if cls is None:
                continue
            Kc = Khi if cls == 0 else Klo
            for r in range(2):
                p = slot + r
                for m in range(2):
                    dz = p - m
                    if 0 <= dz < 5:
                        wk[64 * r : 64 * r + 64, g, t, 64 * m : 64 * m + 64] = Kc[
                            :, :, dz, dy, dx
                        ].T
    return np.ascontiguousarray(wk.astype(E4))


def _pack_x(x):
    """x [1,64,56,56,56] -> per-core fp8 hi/lo slabs:
    (hi [64,10,56,56], lo [64,10,56,56], hi2 [64,6,17,56], lo2 [...])."""
    xs = x[0] * SX
    xhi = xs.astype(E4)
    xlo = (xs - xhi.astype(np.float32)).astype(E4)
    slabs = []
    for c in range(N_CORES):
        z0, p2, y2 = _core_assign(c)
        slabs.append((
            np.ascontiguousarray(xhi[:, z0 : z0 + D_DRAM]),
            np.ascontiguousarray(xlo[:, z0 : z0 + D_DRAM]),
            np.ascontiguousarray(xhi[:, p2 : p2 + P2_DRAM, y2 : y2 + P2_IN_LINES]),
            np.ascontiguousarray(xlo[:, p2 : p2 + P2_DRAM, y2 : y2 + P2_IN_LINES]),
        ))
    return slabs


# ------------------------------------------------------- device program
def build_nc(n_pairs=N_PAIRS, partial=True, repeat=1):
    fp8 = mybir.dt.float8e4
    fp32 = mybir.dt.float32
    nc = bacc.Bacc("TRN2", target_bir_lowering=False, debug=False,
                   num_devices=N_CORES)
    xh_d = nc.dram_tensor("xh", [64, D_DRAM, S, S], fp8, kind="ExternalInput").ap()
    xl_d = nc.dram_tensor("xl", [64, D_DRAM, S, S], fp8, kind="ExternalInput").ap()
    xh2_d = nc.dram_tensor("xh2", [64, P2_DRAM, P2_IN_LINES, S], fp8,
                           kind="ExternalInput").ap()
    xl2_d = nc.dram_tensor("xl2", [64, P2_DRAM, P2_IN_LINES, S], fp8,
                           kind="ExternalInput").ap()
    w_d = nc.dram_tensor("w", [128, NG, 2, 128], fp8, kind="ExternalInput").ap()
    # outputs are plane-major so one DMA can write both planes of a pair:
    # SBUF partitions (z c) = plane-half * 64 + channel
    o_d = nc.dram_tensor("out", [2 * n_pairs, 64, SO, SO], fp32,
                         kind="ExternalOutput").ap()
    o2_d = nc.dram_tensor("out2", [2, 64, P2_LINES, SO], fp32,
                          kind="ExternalOutput").ap()

    DR = mybir.MatmulPerfMode.DoubleRow

    with tile.TileContext(nc) as tc:
        with (
            tc.tile_pool(name="const", bufs=1) as cpool,
            tc.tile_pool(name="outp", bufs=3) as opool,
            tc.tile_pool(name="psum", bufs=8, space="PSUM") as ppool,
        ):
            xh = cpool.tile([128, D_SB, S, S], fp8)
            xl = cpool.tile([128, D_SB, S, S], fp8)
            xh2 = cpool.tile([128, P2_SB, P2_IN_LINES, S], fp8)
            xl2 = cpool.tile([128, P2_SB, P2_IN_LINES, S], fp8)
            wt = cpool.tile([128, NG, 2, 128], fp8)
            # DMA order = first-use order, split across BOTH HWDGE queues
            # (SP = nc.sync, Activation = nc.scalar).  Lead DMAs cover only
            # what the first chunk's phase-0 groups touch (partial slab
            # slots 0..3, lines 0..10) so the first matmul fires early.
            # Dual z-shifted SBUF copies: partitions 0..63 plane j <- plane
            # j, partitions 64..127 plane j <- plane j+1.  Weight slices are
            # interleaved 2:1 ahead of the x planes: fp8 matmuls drain a
            # weight slice (3 groups) every ~260ns, faster than a 1:1
            # interleave can supply them.
            nc.sync.dma_start(wt[:, 0:3], w_d[:, 0:3])
            nc.sync.dma_start(xh2[:64, :4, :11], xh2_d[:, :4, :11])
            nc.scalar.dma_start(xh2[64:, :4, :11], xh2_d[:, 1:5, :11])
            nc.sync.dma_start(xh2[:64, :4, 11:], xh2_d[:, :4, 11:])
            nc.scalar.dma_start(xh2[64:, :4, 11:], xh2_d[:, 1:5, 11:])
            nc.sync.dma_start(xh2[:64, 4], xh2_d[:, 4])
            nc.scalar.dma_start(xh2[64:, 4], xh2_d[:, 5])
            nc.scalar.dma_start(xl2[:64], xl2_d[:, :P2_SB])
            nc.scalar.dma_start(xl2[64:], xl2_d[:, 1 : P2_SB + 1])

            wops = [
                (lambda i=i, eng=eng: eng.dma_start(
                    wt[:, 3 * i : 3 * (i + 1)], w_d[:, 3 * i : 3 * (i + 1)]))
                for i in range(1, NG // 3)
                for eng in ((nc.sync,) if i % 2 else (nc.scalar,))
            ]
            sync_x = []
            scal_x = []
            for j in range(D_SB):
                sync_x.append(lambda j=j: nc.sync.dma_start(xh[:64, j], xh_d[:, j]))
                sync_x.append(
                    lambda j=j: nc.sync.dma_start(xh[64:, j], xh_d[:, j + 1])
                )
                scal_x.append(lambda j=j: nc.scalar.dma_start(xl[:64, j], xl_d[:, j]))
                scal_x.append(
                    lambda j=j: nc.scalar.dma_start(xl[64:, j], xl_d[:, j + 1])
                )
            while wops or sync_x or scal_x:
                for _ in range(2):
                    if wops:
                        wops.pop(0)()
                if sync_x:
                    sync_x.pop(0)()
                if scal_x:
                    scal_x.pop(0)()

            def do_chunk(hi, lo, z, ys, L, dst, zo):
                ps = ppool.tile([128, 9, SO], fp32)
                for g, (dy, dx, kind, sA, sB, cA, cB) in enumerate(GROUPS):
                    src = hi if kind == 0 else lo
                    rhs = src[:, z + sA : z + sB + 1 : (sB - sA),
                              ys + dy : ys + dy + L, dx : dx + SO]
                    nc.tensor.matmul(ps[:, :L, :], wt[:, g], rhs,
                                     start=(g == 0), stop=(g == NG - 1),
                                     perf_mode=DR)
                ot = opool.tile([128, 9, SO], fp32)
                nc.vector.tensor_copy(ot[:, :L], ps[:, :L])
                dst2 = dst[zo : zo + 2, :, ys : ys + L, :].rearrange(
                    "z c l w -> (z c) l w"
                )
                nc.sync.dma_start(dst2, ot[:, :L])

            for _ in range(repeat):
                # partial first: its input lands quickly, hiding the main
                # slab's DMA behind the partial block's compute
                if partial:
                    for ys, L in CHUNKS2:
                        do_chunk(xh2, xl2, 0, ys, L, o2_d, 0)
                for p in range(n_pairs):
                    chunks = CHUNKS_LAST if p == n_pairs - 1 else CHUNKS
                    for ys, L in chunks:
                        do_chunk(xh, xl, 2 * p, ys, L, o_d, 2 * p)
    nc.compile()
    return nc


# ------------------------------------------------------------ entry point
LAST_RESULTS = None
LAST_NC = None
LAST_INMAPS = None


def kernel(x, weight, w_sc0, w_sc1):
    global LAST_RESULTS, LAST_NC, LAST_INMAPS
    x = np.asarray(x, dtype=np.float32)
    K = _fold_self_connection(
        _make_kernel(np.asarray(weight, dtype=np.float32)),
        np.asarray(w_sc0, dtype=np.float32),
        np.asarray(w_sc1, dtype=np.float32),
    )
    wk = _pack_weights(K)
    slabs = _pack_x(x)

    repeat = int(os.environ.get("KERNEL_REPEAT", "1"))
    nc = build_nc(repeat=repeat)
    in_maps = [
        {"xh": slabs[c][0], "xl": slabs[c][1], "xh2": slabs[c][2],
         "xl2": slabs[c][3], "w": wk}
        for c in range(N_CORES)
    ]
    res = run_bass_kernel_spmd(nc, in_maps, core_ids=list(range(N_CORES)))
    LAST_RESULTS, LAST_NC, LAST_INMAPS = res, nc, in_maps

    full = np.zeros((1, 64, SO, SO, SO), np.float32)
    for c in range(N_CORES):
        z0, p2, y2 = _core_assign(c)
        # device outputs are plane-major [z, c, l, w]
        full[0, :, z0 : z0 + D_OUT] = res.results[c]["out"].transpose(1, 0, 2, 3)
        full[0, :, p2 : p2 + 2, y2 : y2 + P2_LINES, :] = res.results[c][
            "out2"
        ].transpose(1, 0, 2, 3)
    return full * OUT_SCALE


# revision 7
# speedup vs baseline: 1.0112x; 1.0112x over previous
"""Trainium2 Bass kernel for nn_Convolution_1176821039998.

Equivariant (e3nn-style) 3D convolution, kernel 5x5x5, 64->64 channels, on a
[1,64,56,56,56] fp32 volume, plus a per-irrep self-connection on the cropped
volume.  Strategy:

Host side (tiny, fp32):
  - Build the dense conv kernel K[o,i,dz,dy,dx] from the TP weight exactly as
    the reference does, and fold the self-connection into the center tap.
  - Perfectly balanced z-shard across 8 cores with NO redundant compute:
    core c computes 6 "main" output planes 6c..6c+5 (planes 0..47) plus a
    13-line y-block of one of the remaining 2 plane-pairs (planes 48..51,
    block chosen by core index).  The partial block's input sub-volume is
    packed by the host into a small side tensor at FIXED local coordinates,
    so all cores run the identical SPMD program (3.25 plane-pairs each).
  - The kernel halo (4 planes) is handled by overlapping shards; no
    device-to-device exchange.

Zero-tap structure: the equivariant kernel is EXACTLY zero for taps with
lattice distance >= 1.25 (the smooth_finite radial embedding vanishes):
  K[:,:,dz,dy,dx] == 0  iff  (dz-2)^2+(dy-2)^2+(dx-2)^2 >= 7.
Per (dy,dx) column with rho2=(dy-2)^2+(dx-2)^2: rho2<=2 (9 cols) keeps all
5 dz taps; rho2 in {4,5} (12 cols) keeps only dz in {1,2,3}; the 4 corner
columns are fully zero and skipped.

Device side (fp8e4m3 DoubleRow matmuls, 0.5 PE cycles per output column):
  - Operands are split hi/lo: xs=8*x -> xhi=e4m3(xs), xlo=e4m3(xs-xhi);
    Ks=32*K -> Khi, Klo likewise.  The product is computed as
    Khi*xhi + Klo*xhi + Khi*xlo (the Klo*xlo term is dropped; measured
    max-rel-err of the scheme vs fp32 reference is ~2e-3).  Host divides
    the gathered output by 256.
  - SBUF holds dual z-shifted copies (partitions 0..63 plane j, 64..127
    plane j+1) of the hi and lo volumes.  A DoubleRow matmul contracts TWO
    such k-tiles (at a constant plane stride, expressed as a strided slice)
    against a [128, 2, 128] fp8 weight pair, accumulating 4 plane-taps per
    instruction into a PSUM pair of output planes (M = 64 ch x 2 planes).
  - Per (dy,dx) column and plane-pair: rho2<=2 needs k-tile slots
    {0,0,2,2,4,4} (hi, classes hi/lo per plane) + {0,2,4} (lo volume, hi
    class) -> 5 DR matmuls; rho2 in {4,5}: slots {1,1,3,3} + {1,3} -> 3.
    81 DR matmuls per chunk (vs 51 fp16 matmuls = 2x fewer PE cycles).
"""

import os
import numpy as np
import ml_dtypes

import concourse.bass as bass
import concourse.mybir as mybir
import concourse.tile as tile
from concourse import bacc
from concourse.bass_utils import run_bass_kernel_spmd

# ---------------------------------------------------------------- constants
SIZE = 5
MUL = 16
CROP = SIZE // 2
PW0 = np.float32((1.0 / 32.0) ** 0.5)
PW1 = np.float32((3.0 / 32.0) ** 0.5)
INV_SQRT3 = np.float32(3.0 ** -0.5)

N_CORES = 8
S = 56                                 # input spatial size
SO = 52                                # output spatial size
# main shard: 3 pairs = 6 output planes at z0 = 6c (covers 0..47)
N_PAIRS = 3
D_OUT = 6
D_DRAM = 10                            # main DRAM slab planes (6c..6c+9)
D_SB = 9                               # planes per SBUF copy (lo 0..8, hi 1..9)
# partial shard: planes 48..51 split into 8 (pair, 13-line block) quarters
P2_BASE = 48
P2_LINES = 13
P2_IN_LINES = P2_LINES + 4             # 17
P2_DRAM = 6                            # partial DRAM planes (P..P+5)
P2_SB = 5                              # partial SBUF planes per copy
CHUNKS = [(0, 9), (9, 9), (18, 9), (27, 9), (36, 9), (45, 7)]  # (y0, lines)
# last pair ends with a tiny chunk so the final PSUM-evict + store DMA tail
# after the last matmul is short
CHUNKS_LAST = [(0, 9), (9, 9), (18, 9), (27, 9), (36, 9), (45, 5), (50, 2)]
CHUNKS2 = [(0, 7), (7, 6)]             # partial 13-line block

E4 = ml_dtypes.float8_e4m3fn
SX = np.float32(8.0)                   # x scale before fp8 quantization
SK = np.float32(32.0)                  # K scale before fp8 quantization
OUT_SCALE = np.float32(1.0 / (8.0 * 32.0))


def _dr_groups():
    """DoubleRow matmul groups per (chunk x plane-pair).

    Each entry: (dy, dx, kind, slotA, slotB, clsA, clsB).
    kind 0 = hi volume, 1 = lo volume.  slot = dual-copy slot relative to the
    pair base z (slot j holds planes z+j / z+j+1 in its two partition
    halves).  cls 0 = Khi, 1 = Klo, None = zero-weight padding half.
    Ordered in phases so early groups only touch low slots (DMA gating).
    """
    cols = [(dy, dx) for dy in range(5) for dx in range(5)
            if (dy - 2) ** 2 + (dx - 2) ** 2 <= 5]

    def full(dy, dx):
        return (dy - 2) ** 2 + (dx - 2) ** 2 <= 2

    phases = ([], [], [], [], [])
    for dy, dx in cols:
        if full(dy, dx):
            phases[0].append((dy, dx, 0, 0, 2, 0, 0))
            phases[1].append((dy, dx, 0, 2, 4, 1, 0))
            phases[2].append((dy, dx, 0, 0, 4, 1, 1))
            phases[3].append((dy, dx, 1, 0, 2, 0, 0))
            # the odd lo k-tile pairs with a free slot; use it for a partial
            # 4th-term (Klo*xlo, planes 2..3) correction at zero extra cost
            phases[4].append((dy, dx, 1, 2, 4, 1, 0))
        else:
            phases[0].append((dy, dx, 0, 1, 3, 0, 0))
            phases[1].append((dy, dx, 0, 1, 3, 1, 1))
            phases[3].append((dy, dx, 1, 1, 3, 0, 0))
    return [g for ph in phases for g in ph]


GROUPS = _dr_groups()
NG = len(GROUPS)  # 81


def _core_assign(c):
    """(main z0, partial pair base, partial y0) for core c."""
    return 6 * c, P2_BASE + 2 * (c // 4), P2_LINES * (c % 4)


# ------------------------------------------------------- host-side weights
def _lattice_consts():
    r = np.linspace(-1.0, 1.0, SIZE, dtype=np.float32)
    lat = np.stack(np.meshgrid(r, r, r, indexing="ij"), axis=-1)
    d = np.linalg.norm(lat.astype(np.float64), axis=-1).astype(np.float32)
    values = np.linspace(0.0, 1.0, SIZE, dtype=np.float32)
    step = values[1] - values[0]
    diff = (d[..., None] - values) / step

    def sus(t):
        return np.where(t > 0, np.exp(-1.0 / np.where(t > 0, t, 1.0)), 0.0).astype(
            np.float32
        )

    emb = np.float32(1.14136) * np.float32(np.e ** 2) * sus(diff + 1.0) * sus(1.0 - diff)
    n = lat / np.maximum(d, 1e-12)[..., None]
    sh0 = np.ones_like(d)
    sh1 = np.float32(3.0 ** 0.5) * n
    return emb.astype(np.float32), sh0, sh1.astype(np.float32)


def _make_kernel(weight):
    """[5,1024] -> conv kernel [out=64, in=64, 5,5,5] fp32 (mirrors reference)."""
    emb, sh0, sh1 = _lattice_consts()
    w = emb @ weight
    Ssp = w.shape[:3]
    blk = MUL * MUL
    w1, w2, w3, w4 = [
        w[..., i * blk : (i + 1) * blk].reshape(*Ssp, MUL, MUL) for i in range(4)
    ]
    k_ss = PW0 * w1 * sh0[..., None, None]
    k_sv = PW1 * INV_SQRT3 * np.einsum("...uw,...k->...uwk", w2, sh1)
    k_vs = PW0 * INV_SQRT3 * np.einsum("...uw,...i->...uiw", w4, sh1)
    eye3 = np.eye(3, dtype=w.dtype)
    k_vv = (
        PW1
        * INV_SQRT3
        * (w3 * sh0[..., None, None])[..., :, None, :, None]
        * eye3[None, None, None, None, :, None, :]
    )
    top = np.concatenate([k_ss, k_sv.reshape(*Ssp, MUL, 3 * MUL)], axis=-1)
    bot = np.concatenate(
        [k_vs.reshape(*Ssp, 3 * MUL, MUL), k_vv.reshape(*Ssp, 3 * MUL, 3 * MUL)],
        axis=-1,
    )
    kernel = np.concatenate([top, bot], axis=-2)  # [5,5,5,in,out]
    return np.ascontiguousarray(np.transpose(kernel, (4, 3, 0, 1, 2)))


def _fold_self_connection(K, w_sc0, w_sc1):
    """Add the cropped e3nn Linear self-connection into the center tap."""
    inv = np.float32(1.0 / MUL ** 0.5)
    sc = np.zeros((64, 64), np.float32)
    sc[:MUL, :MUL] = w_sc0.T * inv  # sc[out w, in u] = w_sc0[u, w]
    for wo in range(MUL):
        for u in range(MUL):
            for k in range(3):
                sc[MUL + 3 * wo + k, MUL + 3 * u + k] += w_sc1[u, wo] * inv
    K = K.copy()
    K[:, :, CROP, CROP, CROP] += sc
    return K


def _pack_weights(K):
    """[64,64,5,5,5] fp32 -> DoubleRow lhsT tiles [128, NG, 2, 128] fp8.

    Row-half r of k-tile t holds the blocks for plane slot+r; column half m
    (out plane z+m) holds tap dz = plane - m from Khi or Klo per the group's
    class assignment."""
    Ks = K * SK
    Khi = Ks.astype(E4).astype(np.float32)
    Klo = (Ks - Khi).astype(E4).astype(np.float32)
    wk = np.zeros((128, NG, 2, 128), np.float32)
    for g, (dy, dx, kind, sA, sB, cA, cB) in enumerate(GROUPS):
        for t, (slot, cls) in enumerate(((sA, cA), (sB, cB))):
            if cls is None:
                continue
            Kc = Khi if cls == 0 else Klo
            for r in range(2):
                p = slot + r
                for m in range(2):
                    dz = p - m
                    if 0 <= dz < 5:
                        wk[64 * r : 64 * r + 64, g, t, 64 * m : 64 * m + 64] = Kc[
                            :, :, dz, dy, dx
                        ].T
    return np.ascontiguousarray(wk.astype(E4))


def _pack_x(x):
    """x [1,64,56,56,56] -> per-core fp8 hi/lo slabs:
    (hi [64,10,56,56], lo [64,10,56,56], hi2 [64,6,17,56], lo2 [...])."""
    xs = x[0] * SX
    xhi = xs.astype(E4)
    xlo = (xs - xhi.astype(np.float32)).astype(E4)
    slabs = []
    for c in range(N_CORES):
        z0, p2, y2 = _core_assign(c)
        slabs.append((
            np.ascontiguousarray(xhi[:, z0 : z0 + D_DRAM]),
            np.ascontiguousarray(xlo[:, z0 : z0 + D_DRAM]),
            np.ascontiguousarray(xhi[:, p2 : p2 + P2_DRAM, y2 : y2 + P2_IN_LINES]),
            np.ascontiguousarray(xlo[:, p2 : p2 + P2_DRAM, y2 : y2 + P2_IN_LINES]),
        ))
    return slabs


# ------------------------------------------------------- device program
def build_nc(n_pairs=N_PAIRS, partial=True, repeat=1):
    fp8 = mybir.dt.float8e4
    fp32 = mybir.dt.float32
    nc = bacc.Bacc("TRN2", target_bir_lowering=False, debug=False,
                   num_devices=N_CORES)
    xh_d = nc.dram_tensor("xh", [64, D_DRAM, S, S], fp8, kind="ExternalInput").ap()
    xl_d = nc.dram_tensor("xl", [64, D_DRAM, S, S], fp8, kind="ExternalInput").ap()
    xh2_d = nc.dram_tensor("xh2", [64, P2_DRAM, P2_IN_LINES, S], fp8,
                           kind="ExternalInput").ap()
    xl2_d = nc.dram_tensor("xl2", [64, P2_DRAM, P2_IN_LINES, S], fp8,
                           kind="ExternalInput").ap()
    w_d = nc.dram_tensor("w", [128, NG, 2, 128], fp8, kind="ExternalInput").ap()
    # outputs are plane-major so one DMA can write both planes of a pair:
    # SBUF partitions (z c) = plane-half * 64 + channel
    o_d = nc.dram_tensor("out", [2 * n_pairs, 64, SO, SO], fp32,
                         kind="ExternalOutput").ap()
    o2_d = nc.dram_tensor("out2", [2, 64, P2_LINES, SO], fp32,
                          kind="ExternalOutput").ap()

    DR = mybir.MatmulPerfMode.DoubleRow

    with tile.TileContext(nc) as tc:
        with (
            tc.tile_pool(name="const", bufs=1) as cpool,
            tc.tile_pool(name="outp", bufs=3) as opool,
            tc.tile_pool(name="psum", bufs=8, space="PSUM") as ppool,
        ):
            xh = cpool.tile([128, D_SB, S, S], fp8)
            xl = cpool.tile([128, D_SB, S, S], fp8)
            xh2 = cpool.tile([128, P2_SB, P2_IN_LINES, S], fp8)
            xl2 = cpool.tile([128, P2_SB, P2_IN_LINES, S], fp8)
            wt = cpool.tile([128, NG, 2, 128], fp8)
            # DMA order = first-use order, split across BOTH HWDGE queues
            # (SP = nc.sync, Activation = nc.scalar).  Lead DMAs cover only
            # what the first chunk's phase-0 groups touch (partial slab
            # slots 0..3, lines 0..10) so the first matmul fires early.
            # Dual z-shifted SBUF copies: partitions 0..63 plane j <- plane
            # j, partitions 64..127 plane j <- plane j+1.  Weight slices are
            # interleaved 2:1 ahead of the x planes: fp8 matmuls drain a
            # weight slice (3 groups) every ~260ns, faster than a 1:1
            # interleave can supply them.
            nc.sync.dma_start(wt[:, 0:3], w_d[:, 0:3])
            nc.sync.dma_start(xh2[:64, :4, :11], xh2_d[:, :4, :11])
            nc.scalar.dma_start(xh2[64:, :4, :11], xh2_d[:, 1:5, :11])
            nc.sync.dma_start(xh2[:64, :4, 11:], xh2_d[:, :4, 11:])
            nc.scalar.dma_start(xh2[64:, :4, 11:], xh2_d[:, 1:5, 11:])
            nc.sync.dma_start(xh2[:64, 4], xh2_d[:, 4])
            nc.scalar.dma_start(xh2[64:, 4], xh2_d[:, 5])
            nc.scalar.dma_start(xl2[:64], xl2_d[:, :P2_SB])
            nc.scalar.dma_start(xl2[64:], xl2_d[:, 1 : P2_SB + 1])

            wops = [
                (lambda i=i, eng=eng: eng.dma_start(
                    wt[:, 3 * i : 3 * (i + 1)], w_d[:, 3 * i : 3 * (i + 1)]))
                for i in range(1, NG // 3)
                for eng in ((nc.sync,) if i % 2 else (nc.scalar,))
            ]
            sync_x = []
            scal_x = []
            for j in range(D_SB):
                sync_x.append(lambda j=j: nc.sync.dma_start(xh[:64, j], xh_d[:, j]))
                sync_x.append(
                    lambda j=j: nc.sync.dma_start(xh[64:, j], xh_d[:, j + 1])
                )
                scal_x.append(lambda j=j: nc.scalar.dma_start(xl[:64, j], xl_d[:, j]))
                scal_x.append(
                    lambda j=j: nc.scalar.dma_start(xl[64:, j], xl_d[:, j + 1])
                )
            while wops or sync_x or scal_x:
                if wops:
                    wops.pop(0)()
                if sync_x:
                    sync_x.pop(0)()
                if scal_x:
                    scal_x.pop(0)()

            def do_chunk(hi, lo, z, ys, L, dst, zo):
                ps = ppool.tile([128, 9, SO], fp32)
                for g, (dy, dx, kind, sA, sB, cA, cB) in enumerate(GROUPS):
                    src = hi if kind == 0 else lo
                    rhs = src[:, z + sA : z + sB + 1 : (sB - sA),
                              ys + dy : ys + dy + L, dx : dx + SO]
                    nc.tensor.matmul(ps[:, :L, :], wt[:, g], rhs,
                                     start=(g == 0), stop=(g == NG - 1),
                                     perf_mode=DR)
                ot = opool.tile([128, 9, SO], fp32)
                nc.vector.tensor_copy(ot[:, :L], ps[:, :L])
                dst2 = dst[zo : zo + 2, :, ys : ys + L, :].rearrange(
                    "z c l w -> (z c) l w"
                )
                nc.sync.dma_start(dst2, ot[:, :L])

            for _ in range(repeat):
                # partial first: its input lands quickly, hiding the main
                # slab's DMA behind the partial block's compute
                if partial:
                    for ys, L in CHUNKS2:
                        do_chunk(xh2, xl2, 0, ys, L, o2_d, 0)
                for p in range(n_pairs):
                    chunks = CHUNKS_LAST if p == n_pairs - 1 else CHUNKS
                    for ys, L in chunks:
                        do_chunk(xh, xl, 2 * p, ys, L, o_d, 2 * p)
    nc.compile()
    return nc


# ------------------------------------------------------------ entry point
LAST_RESULTS = None
LAST_NC = None
LAST_INMAPS = None


def kernel(x, weight, w_sc0, w_sc1):
    global LAST_RESULTS, LAST_NC, LAST_INMAPS
    x = np.asarray(x, dtype=np.float32)
    K = _fold_self_connection(
        _make_kernel(np.asarray(weight, dtype=np.float32)),
        np.asarray(w_sc0, dtype=np.float32),
        np.asarray(w_sc1, dtype=np.float32),
    )
    wk = _pack_weights(K)
    slabs = _pack_x(x)

    repeat = int(os.environ.get("KERNEL_REPEAT", "1"))
    nc = build_nc(repeat=repeat)
    in_maps = [
        {"xh": slabs[c][0], "xl": slabs[c][1], "xh2": slabs[c][2],
         "xl2": slabs[c][3], "w": wk}
        for c in range(N_CORES)
    ]
    res = run_bass_kernel_spmd(nc, in_maps, core_ids=list(range(N_CORES)))
    LAST_RESULTS, LAST_NC, LAST_INMAPS = res, nc, in_maps

    full = np.zeros((1, 64, SO, SO, SO), np.float32)
    for c in range(N_CORES):
        z0, p2, y2 = _core_assign(c)
        # device outputs are plane-major [z, c, l, w]
        full[0, :, z0 : z0 + D_OUT] = res.results[c]["out"].transpose(1, 0, 2, 3)
        full[0, :, p2 : p2 + 2, y2 : y2 + P2_LINES, :] = res.results[c][
            "out2"
        ].transpose(1, 0, 2, 3)
    return full * OUT_SCALE


# revision 8
# speedup vs baseline: 1.0124x; 1.0013x over previous
"""Trainium2 Bass kernel for nn_Convolution_1176821039998.

Equivariant (e3nn-style) 3D convolution, kernel 5x5x5, 64->64 channels, on a
[1,64,56,56,56] fp32 volume, plus a per-irrep self-connection on the cropped
volume.  Strategy:

Host side (tiny, fp32):
  - Build the dense conv kernel K[o,i,dz,dy,dx] from the TP weight exactly as
    the reference does, and fold the self-connection into the center tap.
  - Perfectly balanced z-shard across 8 cores with NO redundant compute:
    core c computes 6 "main" output planes 6c..6c+5 (planes 0..47) plus a
    13-line y-block of one of the remaining 2 plane-pairs (planes 48..51,
    block chosen by core index).  The partial block's input sub-volume is
    packed by the host into a small side tensor at FIXED local coordinates,
    so all cores run the identical SPMD program (3.25 plane-pairs each).
  - The kernel halo (4 planes) is handled by overlapping shards; no
    device-to-device exchange.

Zero-tap structure: the equivariant kernel is EXACTLY zero for taps with
lattice distance >= 1.25 (the smooth_finite radial embedding vanishes):
  K[:,:,dz,dy,dx] == 0  iff  (dz-2)^2+(dy-2)^2+(dx-2)^2 >= 7.
Per (dy,dx) column with rho2=(dy-2)^2+(dx-2)^2: rho2<=2 (9 cols) keeps all
5 dz taps; rho2 in {4,5} (12 cols) keeps only dz in {1,2,3}; the 4 corner
columns are fully zero and skipped.

Device side (fp8e4m3 DoubleRow matmuls, 0.5 PE cycles per output column):
  - Operands are split hi/lo: xs=8*x -> xhi=e4m3(xs), xlo=e4m3(xs-xhi);
    Ks=32*K -> Khi, Klo likewise.  The product is computed as
    Khi*xhi + Klo*xhi + Khi*xlo (the Klo*xlo term is dropped; measured
    max-rel-err of the scheme vs fp32 reference is ~2e-3).  Host divides
    the gathered output by 256.
  - SBUF holds dual z-shifted copies (partitions 0..63 plane j, 64..127
    plane j+1) of the hi and lo volumes.  A DoubleRow matmul contracts TWO
    such k-tiles (at a constant plane stride, expressed as a strided slice)
    against a [128, 2, 128] fp8 weight pair, accumulating 4 plane-taps per
    instruction into a PSUM pair of output planes (M = 64 ch x 2 planes).
  - Per (dy,dx) column and plane-pair: rho2<=2 needs k-tile slots
    {0,0,2,2,4,4} (hi, classes hi/lo per plane) + {0,2,4} (lo volume, hi
    class) -> 5 DR matmuls; rho2 in {4,5}: slots {1,1,3,3} + {1,3} -> 3.
    81 DR matmuls per chunk (vs 51 fp16 matmuls = 2x fewer PE cycles).
"""

import os
import numpy as np
import ml_dtypes

import concourse.bass as bass
import concourse.mybir as mybir
import concourse.tile as tile
from concourse import bacc
from concourse.bass_utils import run_bass_kernel_spmd

# ---------------------------------------------------------------- constants
SIZE = 5
MUL = 16
CROP = SIZE // 2
PW0 = np.float32((1.0 / 32.0) ** 0.5)
PW1 = np.float32((3.0 / 32.0) ** 0.5)
INV_SQRT3 = np.float32(3.0 ** -0.5)

N_CORES = 8
S = 56                                 # input spatial size
SO = 52                                # output spatial size
# main shard: 3 pairs = 6 output planes at z0 = 6c (covers 0..47)
N_PAIRS = 3
D_OUT = 6
D_DRAM = 10                            # main DRAM slab planes (6c..6c+9)
D_SB = 9                               # planes per SBUF copy (lo 0..8, hi 1..9)
# partial shard: planes 48..51 split into 8 (pair, 13-line block) quarters
P2_BASE = 48
P2_LINES = 13
P2_IN_LINES = P2_LINES + 4             # 17
P2_DRAM = 6                            # partial DRAM planes (P..P+5)
P2_SB = 5                              # partial SBUF planes per copy
CHUNKS = [(0, 9), (9, 9), (18, 9), (27, 9), (36, 9), (45, 7)]  # (y0, lines)
# last pair ends with a tiny chunk so the final PSUM-evict + store DMA tail
# after the last matmul is short
CHUNKS_LAST = [(0, 9), (9, 9), (18, 9), (27, 9), (36, 9), (45, 5), (50, 2)]
CHUNKS2 = [(0, 7), (7, 6)]             # partial 13-line block

E4 = ml_dtypes.float8_e4m3fn
SX = np.float32(8.0)                   # x scale before fp8 quantization
SK = np.float32(32.0)                  # K scale before fp8 quantization
OUT_SCALE = np.float32(1.0 / (8.0 * 32.0))


def _dr_groups():
    """DoubleRow matmul groups per (chunk x plane-pair).

    Each entry: (dy, dx, kind, slotA, slotB, clsA, clsB).
    kind 0 = hi volume, 1 = lo volume.  slot = dual-copy slot relative to the
    pair base z (slot j holds planes z+j / z+j+1 in its two partition
    halves).  cls 0 = Khi, 1 = Klo, None = zero-weight padding half.
    Ordered in phases so early groups only touch low slots (DMA gating).
    """
    cols = [(dy, dx) for dy in range(5) for dx in range(5)
            if (dy - 2) ** 2 + (dx - 2) ** 2 <= 5]

    def full(dy, dx):
        return (dy - 2) ** 2 + (dx - 2) ** 2 <= 2

    phases = ([], [], [], [], [])
    for dy, dx in cols:
        if full(dy, dx):
            phases[0].append((dy, dx, 0, 0, 2, 0, 0))
            phases[1].append((dy, dx, 0, 2, 4, 1, 0))
            phases[2].append((dy, dx, 0, 0, 4, 1, 1))
            phases[3].append((dy, dx, 1, 0, 2, 0, 0))
            # the odd lo k-tile pairs with a free slot; use it for a partial
            # 4th-term (Klo*xlo, planes 2..3) correction at zero extra cost
            phases[4].append((dy, dx, 1, 2, 4, 1, 0))
        else:
            phases[0].append((dy, dx, 0, 1, 3, 0, 0))
            phases[1].append((dy, dx, 0, 1, 3, 1, 1))
            phases[3].append((dy, dx, 1, 1, 3, 0, 0))
    return [g for ph in phases for g in ph]


GROUPS = _dr_groups()
NG = len(GROUPS)  # 81


def _core_assign(c):
    """(main z0, partial pair base, partial y0) for core c."""
    return 6 * c, P2_BASE + 2 * (c // 4), P2_LINES * (c % 4)


# ------------------------------------------------------- host-side weights
def _lattice_consts():
    r = np.linspace(-1.0, 1.0, SIZE, dtype=np.float32)
    lat = np.stack(np.meshgrid(r, r, r, indexing="ij"), axis=-1)
    d = np.linalg.norm(lat.astype(np.float64), axis=-1).astype(np.float32)
    values = np.linspace(0.0, 1.0, SIZE, dtype=np.float32)
    step = values[1] - values[0]
    diff = (d[..., None] - values) / step

    def sus(t):
        return np.where(t > 0, np.exp(-1.0 / np.where(t > 0, t, 1.0)), 0.0).astype(
            np.float32
        )

    emb = np.float32(1.14136) * np.float32(np.e ** 2) * sus(diff + 1.0) * sus(1.0 - diff)
    n = lat / np.maximum(d, 1e-12)[..., None]
    sh0 = np.ones_like(d)
    sh1 = np.float32(3.0 ** 0.5) * n
    return emb.astype(np.float32), sh0, sh1.astype(np.float32)


def _make_kernel(weight):
    """[5,1024] -> conv kernel [out=64, in=64, 5,5,5] fp32 (mirrors reference)."""
    emb, sh0, sh1 = _lattice_consts()
    w = emb @ weight
    Ssp = w.shape[:3]
    blk = MUL * MUL
    w1, w2, w3, w4 = [
        w[..., i * blk : (i + 1) * blk].reshape(*Ssp, MUL, MUL) for i in range(4)
    ]
    k_ss = PW0 * w1 * sh0[..., None, None]
    k_sv = PW1 * INV_SQRT3 * np.einsum("...uw,...k->...uwk", w2, sh1)
    k_vs = PW0 * INV_SQRT3 * np.einsum("...uw,...i->...uiw", w4, sh1)
    eye3 = np.eye(3, dtype=w.dtype)
    k_vv = (
        PW1
        * INV_SQRT3
        * (w3 * sh0[..., None, None])[..., :, None, :, None]
        * eye3[None, None, None, None, :, None, :]
    )
    top = np.concatenate([k_ss, k_sv.reshape(*Ssp, MUL, 3 * MUL)], axis=-1)
    bot = np.concatenate(
        [k_vs.reshape(*Ssp, 3 * MUL, MUL), k_vv.reshape(*Ssp, 3 * MUL, 3 * MUL)],
        axis=-1,
    )
    kernel = np.concatenate([top, bot], axis=-2)  # [5,5,5,in,out]
    return np.ascontiguousarray(np.transpose(kernel, (4, 3, 0, 1, 2)))


def _fold_self_connection(K, w_sc0, w_sc1):
    """Add the cropped e3nn Linear self-connection into the center tap."""
    inv = np.float32(1.0 / MUL ** 0.5)
    sc = np.zeros((64, 64), np.float32)
    sc[:MUL, :MUL] = w_sc0.T * inv  # sc[out w, in u] = w_sc0[u, w]
    for wo in range(MUL):
        for u in range(MUL):
            for k in range(3):
                sc[MUL + 3 * wo + k, MUL + 3 * u + k] += w_sc1[u, wo] * inv
    K = K.copy()
    K[:, :, CROP, CROP, CROP] += sc
    return K


def _pack_weights(K):
    """[64,64,5,5,5] fp32 -> DoubleRow lhsT tiles [128, NG, 2, 128] fp8.

    Row-half r of k-tile t holds the blocks for plane slot+r; column half m
    (out plane z+m) holds tap dz = plane - m from Khi or Klo per the group's
    class assignment."""
    Ks = K * SK
    Khi = Ks.astype(E4).astype(np.float32)
    Klo = (Ks - Khi).astype(E4).astype(np.float32)
    wk = np.zeros((128, NG, 2, 128), np.float32)
    for g, (dy, dx, kind, sA, sB, cA, cB) in enumerate(GROUPS):
        for t, (slot, cls) in enumerate(((sA, cA), (sB, cB))):
            if cls is None:
                continue
            Kc = Khi if cls == 0 else Klo
            for r in range(2):
                p = slot + r
                for m in range(2):
                    dz = p - m
                    if 0 <= dz < 5:
                        wk[64 * r : 64 * r + 64, g, t, 64 * m : 64 * m + 64] = Kc[
                            :, :, dz, dy, dx
                        ].T
    return np.ascontiguousarray(wk.astype(E4))


def _pack_x(x):
    """x [1,64,56,56,56] -> per-core fp8 hi/lo slabs:
    (hi [64,10,56,56], lo [64,10,56,56], hi2 [64,6,17,56], lo2 [...])."""
    xs = x[0] * SX
    xhi = xs.astype(E4)
    xlo = (xs - xhi.astype(np.float32)).astype(E4)
    slabs = []
    for c in range(N_CORES):
        z0, p2, y2 = _core_assign(c)
        slabs.append((
            np.ascontiguousarray(xhi[:, z0 : z0 + D_DRAM]),
            np.ascontiguousarray(xlo[:, z0 : z0 + D_DRAM]),
            np.ascontiguousarray(xhi[:, p2 : p2 + P2_DRAM, y2 : y2 + P2_IN_LINES]),
            np.ascontiguousarray(xlo[:, p2 : p2 + P2_DRAM, y2 : y2 + P2_IN_LINES]),
        ))
    return slabs


# ------------------------------------------------------- device program
def build_nc(n_pairs=N_PAIRS, partial=True, repeat=1):
    fp8 = mybir.dt.float8e4
    fp32 = mybir.dt.float32
    nc = bacc.Bacc("TRN2", target_bir_lowering=False, debug=False,
                   num_devices=N_CORES)
    xh_d = nc.dram_tensor("xh", [64, D_DRAM, S, S], fp8, kind="ExternalInput").ap()
    xl_d = nc.dram_tensor("xl", [64, D_DRAM, S, S], fp8, kind="ExternalInput").ap()
    xh2_d = nc.dram_tensor("xh2", [64, P2_DRAM, P2_IN_LINES, S], fp8,
                           kind="ExternalInput").ap()
    xl2_d = nc.dram_tensor("xl2", [64, P2_DRAM, P2_IN_LINES, S], fp8,
                           kind="ExternalInput").ap()
    w_d = nc.dram_tensor("w", [128, NG, 2, 128], fp8, kind="ExternalInput").ap()
    # outputs are plane-major so one DMA can write both planes of a pair:
    # SBUF partitions (z c) = plane-half * 64 + channel
    o_d = nc.dram_tensor("out", [2 * n_pairs, 64, SO, SO], fp32,
                         kind="ExternalOutput").ap()
    o2_d = nc.dram_tensor("out2", [2, 64, P2_LINES, SO], fp32,
                          kind="ExternalOutput").ap()

    DR = mybir.MatmulPerfMode.DoubleRow

    with tile.TileContext(nc) as tc:
        with (
            tc.tile_pool(name="const", bufs=1) as cpool,
            tc.tile_pool(name="outp", bufs=3) as opool,
            tc.tile_pool(name="psum", bufs=8, space="PSUM") as ppool,
        ):
            xh = cpool.tile([128, D_SB, S, S], fp8)
            xl = cpool.tile([128, D_SB, S, S], fp8)
            xh2 = cpool.tile([128, P2_SB, P2_IN_LINES, S], fp8)
            xl2 = cpool.tile([128, P2_SB, P2_IN_LINES, S], fp8)
            wt = cpool.tile([128, NG, 2, 128], fp8)
            # DMA order = first-use order, split across BOTH HWDGE queues
            # (SP = nc.sync, Activation = nc.scalar).  Lead DMAs cover only
            # what the first chunk's phase-0 groups touch (partial slab
            # slots 0..3, lines 0..10) so the first matmul fires early.
            # Dual z-shifted SBUF copies: partitions 0..63 plane j <- plane
            # j, partitions 64..127 plane j <- plane j+1.  Weight slices are
            # interleaved 2:1 ahead of the x planes: fp8 matmuls drain a
            # weight slice (3 groups) every ~260ns, faster than a 1:1
            # interleave can supply them.
            nc.sync.dma_start(wt[:, 0:3], w_d[:, 0:3])
            nc.sync.dma_start(xh2[:64], xh2_d[:, :P2_SB])
            nc.scalar.dma_start(xh2[64:], xh2_d[:, 1 : P2_SB + 1])
            nc.sync.dma_start(xl2[:64], xl2_d[:, :P2_SB])
            nc.scalar.dma_start(xl2[64:], xl2_d[:, 1 : P2_SB + 1])

            wops = [
                (lambda i=i, eng=eng: eng.dma_start(
                    wt[:, 3 * i : 3 * (i + 1)], w_d[:, 3 * i : 3 * (i + 1)]))
                for i in range(1, NG // 3)
                for eng in ((nc.sync,) if i % 2 else (nc.scalar,))
            ]
            sync_x = []
            scal_x = []
            for j in range(D_SB):
                sync_x.append(lambda j=j: nc.sync.dma_start(xh[:64, j], xh_d[:, j]))
                sync_x.append(
                    lambda j=j: nc.sync.dma_start(xh[64:, j], xh_d[:, j + 1])
                )
                scal_x.append(lambda j=j: nc.scalar.dma_start(xl[:64, j], xl_d[:, j]))
                scal_x.append(
                    lambda j=j: nc.scalar.dma_start(xl[64:, j], xl_d[:, j + 1])
                )
            while wops or sync_x or scal_x:
                if wops:
                    wops.pop(0)()
                if sync_x:
                    sync_x.pop(0)()
                if scal_x:
                    scal_x.pop(0)()

            def do_chunk(hi, lo, z, ys, L, dst, zo):
                ps = ppool.tile([128, 9, SO], fp32)
                for g, (dy, dx, kind, sA, sB, cA, cB) in enumerate(GROUPS):
                    src = hi if kind == 0 else lo
                    rhs = src[:, z + sA : z + sB + 1 : (sB - sA),
                              ys + dy : ys + dy + L, dx : dx + SO]
                    nc.tensor.matmul(ps[:, :L, :], wt[:, g], rhs,
                                     start=(g == 0), stop=(g == NG - 1),
                                     perf_mode=DR)
                ot = opool.tile([128, 9, SO], fp32)
                nc.vector.tensor_copy(ot[:, :L], ps[:, :L])
                dst2 = dst[zo : zo + 2, :, ys : ys + L, :].rearrange(
                    "z c l w -> (z c) l w"
                )
                nc.sync.dma_start(dst2, ot[:, :L])

            for _ in range(repeat):
                # partial first: its input lands quickly, hiding the main
                # slab's DMA behind the partial block's compute
                if partial:
                    for ys, L in CHUNKS2:
                        do_chunk(xh2, xl2, 0, ys, L, o2_d, 0)
                for p in range(n_pairs):
                    chunks = CHUNKS_LAST if p == n_pairs - 1 else CHUNKS
                    for ys, L in chunks:
                        do_chunk(xh, xl, 2 * p, ys, L, o_d, 2 * p)
    nc.compile()
    return nc


# ------------------------------------------------------------ entry point
LAST_RESULTS = None
LAST_NC = None
LAST_INMAPS = None


def kernel(x, weight, w_sc0, w_sc1):
    global LAST_RESULTS, LAST_NC, LAST_INMAPS
    x = np.asarray(x, dtype=np.float32)
    K = _fold_self_connection(
        _make_kernel(np.asarray(weight, dtype=np.float32)),
        np.asarray(w_sc0, dtype=np.float32),
        np.asarray(w_sc1, dtype=np.float32),
    )
    wk = _pack_weights(K)
    slabs = _pack_x(x)

    repeat = int(os.environ.get("KERNEL_REPEAT", "1"))
    nc = build_nc(repeat=repeat)
    in_maps = [
        {"xh": slabs[c][0], "xl": slabs[c][1], "xh2": slabs[c][2],
         "xl2": slabs[c][3], "w": wk}
        for c in range(N_CORES)
    ]
    res = run_bass_kernel_spmd(nc, in_maps, core_ids=list(range(N_CORES)))
    LAST_RESULTS, LAST_NC, LAST_INMAPS = res, nc, in_maps

    full = np.zeros((1, 64, SO, SO, SO), np.float32)
    for c in range(N_CORES):
        z0, p2, y2 = _core_assign(c)
        # device outputs are plane-major [z, c, l, w]
        full[0, :, z0 : z0 + D_OUT] = res.results[c]["out"].transpose(1, 0, 2, 3)
        full[0, :, p2 : p2 + 2, y2 : y2 + P2_LINES, :] = res.results[c][
            "out2"
        ].transpose(1, 0, 2, 3)
    return full * OUT_SCALE


# revision 9
# speedup vs baseline: 1.0289x; 1.0163x over previous
"""Trainium2 Bass kernel for nn_Convolution_1176821039998.

Equivariant (e3nn-style) 3D convolution, kernel 5x5x5, 64->64 channels, on a
[1,64,56,56,56] fp32 volume, plus a per-irrep self-connection on the cropped
volume.  Strategy:

Host side (tiny, fp32):
  - Build the dense conv kernel K[o,i,dz,dy,dx] from the TP weight exactly as
    the reference does, and fold the self-connection into the center tap.
  - Perfectly balanced z-shard across 8 cores with NO redundant compute:
    core c computes 6 "main" output planes 6c..6c+5 (planes 0..47) plus a
    13-line y-block of one of the remaining 2 plane-pairs (planes 48..51,
    block chosen by core index).  The partial block's input sub-volume is
    packed by the host into a small side tensor at FIXED local coordinates,
    so all cores run the identical SPMD program (3.25 plane-pairs each).
  - The kernel halo (4 planes) is handled by overlapping shards; no
    device-to-device exchange.

Zero-tap structure: the equivariant kernel is EXACTLY zero for taps with
lattice distance >= 1.25 (the smooth_finite radial embedding vanishes):
  K[:,:,dz,dy,dx] == 0  iff  (dz-2)^2+(dy-2)^2+(dx-2)^2 >= 7.
Per (dy,dx) column with rho2=(dy-2)^2+(dx-2)^2: rho2<=2 (9 cols) keeps all
5 dz taps; rho2 in {4,5} (12 cols) keeps only dz in {1,2,3}; the 4 corner
columns are fully zero and skipped.

Device side (fp8e4m3 DoubleRow matmuls, 0.5 PE cycles per output column):
  - Operands are split hi/lo: xs=8*x -> xhi=e4m3(xs), xlo=e4m3(xs-xhi);
    Ks=32*K -> Khi, Klo likewise.  The product is computed as
    Khi*xhi + Klo*xhi + Khi*xlo (the Klo*xlo term is dropped; measured
    max-rel-err of the scheme vs fp32 reference is ~2e-3).  Host divides
    the gathered output by 256.
  - SBUF holds dual z-shifted copies (partitions 0..63 plane j, 64..127
    plane j+1) of the hi and lo volumes.  A DoubleRow matmul contracts TWO
    such k-tiles (at a constant plane stride, expressed as a strided slice)
    against a [128, 2, 128] fp8 weight pair, accumulating 4 plane-taps per
    instruction into a PSUM pair of output planes (M = 64 ch x 2 planes).
  - Per (dy,dx) column and plane-pair: rho2<=2 needs k-tile slots
    {0,0,2,2,4,4} (hi, classes hi/lo per plane) + {0,2,4} (lo volume, hi
    class) -> 5 DR matmuls; rho2 in {4,5}: slots {1,1,3,3} + {1,3} -> 3.
    81 DR matmuls per chunk (vs 51 fp16 matmuls = 2x fewer PE cycles).
"""

import os
import numpy as np
import ml_dtypes

import concourse.bass as bass
import concourse.mybir as mybir
import concourse.tile as tile
from concourse import bacc
from concourse.bass_utils import run_bass_kernel_spmd

# ---------------------------------------------------------------- constants
SIZE = 5
MUL = 16
CROP = SIZE // 2
PW0 = np.float32((1.0 / 32.0) ** 0.5)
PW1 = np.float32((3.0 / 32.0) ** 0.5)
INV_SQRT3 = np.float32(3.0 ** -0.5)

N_CORES = 8
S = 56                                 # input spatial size
SO = 52                                # output spatial size
# main shard: 3 pairs = 6 output planes at z0 = 6c (covers 0..47)
N_PAIRS = 3
D_OUT = 6
D_DRAM = 10                            # main DRAM slab planes (6c..6c+9)
D_SB = 9                               # planes per SBUF copy (lo 0..8, hi 1..9)
# partial shard: planes 48..51 split into 8 (pair, 13-line block) quarters
P2_BASE = 48
P2_LINES = 13
P2_IN_LINES = P2_LINES + 4             # 17
P2_DRAM = 6                            # partial DRAM planes (P..P+5)
P2_SB = 5                              # partial SBUF planes per copy
CHUNKS = [(0, 9), (9, 9), (18, 9), (27, 9), (36, 9), (45, 7)]  # (y0, lines)
# last pair ends with a tiny chunk so the final PSUM-evict + store DMA tail
# after the last matmul is short
CHUNKS_LAST = [(0, 9), (9, 9), (18, 9), (27, 9), (36, 9), (45, 5), (50, 2)]
CHUNKS2 = [(0, 7), (7, 6)]             # partial 13-line block

E4 = ml_dtypes.float8_e4m3fn
SX = np.float32(8.0)                   # x scale before fp8 quantization
SK = np.float32(32.0)                  # K scale before fp8 quantization
OUT_SCALE = np.float32(1.0 / (8.0 * 32.0))


def _dr_groups():
    """DoubleRow matmul groups per (chunk x plane-pair).

    Each entry: (dy, dx, kind, slotA, slotB, clsA, clsB).
    kind 0 = hi volume, 1 = lo volume.  slot = dual-copy slot relative to the
    pair base z (slot j holds planes z+j / z+j+1 in its two partition
    halves).  cls 0 = Khi, 1 = Klo, None = zero-weight padding half.
    Ordered in phases so early groups only touch low slots (DMA gating).
    """
    cols = [(dy, dx) for dy in range(5) for dx in range(5)
            if (dy - 2) ** 2 + (dx - 2) ** 2 <= 5]

    def full(dy, dx):
        return (dy - 2) ** 2 + (dx - 2) ** 2 <= 2

    phases = ([], [], [], [], [])
    for dy, dx in cols:
        if full(dy, dx):
            phases[0].append((dy, dx, 0, 0, 2, 0, 0))
            phases[1].append((dy, dx, 0, 2, 4, 1, 0))
            phases[2].append((dy, dx, 0, 0, 4, 1, 1))
            phases[3].append((dy, dx, 1, 0, 2, 0, 0))
            # the odd lo k-tile pairs with a free slot; use it for a partial
            # 4th-term (Klo*xlo, planes 2..3) correction at zero extra cost
            phases[4].append((dy, dx, 1, 2, 4, 1, 0))
        else:
            phases[0].append((dy, dx, 0, 1, 3, 0, 0))
            phases[1].append((dy, dx, 0, 1, 3, 1, 1))
            phases[3].append((dy, dx, 1, 1, 3, 0, 0))
    return [g for ph in phases for g in ph]


GROUPS = _dr_groups()
NG = len(GROUPS)  # 81


def _core_assign(c):
    """(main z0, partial pair base, partial y0) for core c."""
    return 6 * c, P2_BASE + 2 * (c // 4), P2_LINES * (c % 4)


# ------------------------------------------------------- host-side weights
def _lattice_consts():
    r = np.linspace(-1.0, 1.0, SIZE, dtype=np.float32)
    lat = np.stack(np.meshgrid(r, r, r, indexing="ij"), axis=-1)
    d = np.linalg.norm(lat.astype(np.float64), axis=-1).astype(np.float32)
    values = np.linspace(0.0, 1.0, SIZE, dtype=np.float32)
    step = values[1] - values[0]
    diff = (d[..., None] - values) / step

    def sus(t):
        return np.where(t > 0, np.exp(-1.0 / np.where(t > 0, t, 1.0)), 0.0).astype(
            np.float32
        )

    emb = np.float32(1.14136) * np.float32(np.e ** 2) * sus(diff + 1.0) * sus(1.0 - diff)
    n = lat / np.maximum(d, 1e-12)[..., None]
    sh0 = np.ones_like(d)
    sh1 = np.float32(3.0 ** 0.5) * n
    return emb.astype(np.float32), sh0, sh1.astype(np.float32)


def _make_kernel(weight):
    """[5,1024] -> conv kernel [out=64, in=64, 5,5,5] fp32 (mirrors reference)."""
    emb, sh0, sh1 = _lattice_consts()
    w = emb @ weight
    Ssp = w.shape[:3]
    blk = MUL * MUL
    w1, w2, w3, w4 = [
        w[..., i * blk : (i + 1) * blk].reshape(*Ssp, MUL, MUL) for i in range(4)
    ]
    k_ss = PW0 * w1 * sh0[..., None, None]
    k_sv = PW1 * INV_SQRT3 * np.einsum("...uw,...k->...uwk", w2, sh1)
    k_vs = PW0 * INV_SQRT3 * np.einsum("...uw,...i->...uiw", w4, sh1)
    eye3 = np.eye(3, dtype=w.dtype)
    k_vv = (
        PW1
        * INV_SQRT3
        * (w3 * sh0[..., None, None])[..., :, None, :, None]
        * eye3[None, None, None, None, :, None, :]
    )
    top = np.concatenate([k_ss, k_sv.reshape(*Ssp, MUL, 3 * MUL)], axis=-1)
    bot = np.concatenate(
        [k_vs.reshape(*Ssp, 3 * MUL, MUL), k_vv.reshape(*Ssp, 3 * MUL, 3 * MUL)],
        axis=-1,
    )
    kernel = np.concatenate([top, bot], axis=-2)  # [5,5,5,in,out]
    return np.ascontiguousarray(np.transpose(kernel, (4, 3, 0, 1, 2)))


def _fold_self_connection(K, w_sc0, w_sc1):
    """Add the cropped e3nn Linear self-connection into the center tap."""
    inv = np.float32(1.0 / MUL ** 0.5)
    sc = np.zeros((64, 64), np.float32)
    sc[:MUL, :MUL] = w_sc0.T * inv  # sc[out w, in u] = w_sc0[u, w]
    for wo in range(MUL):
        for u in range(MUL):
            for k in range(3):
                sc[MUL + 3 * wo + k, MUL + 3 * u + k] += w_sc1[u, wo] * inv
    K = K.copy()
    K[:, :, CROP, CROP, CROP] += sc
    return K


def _pack_weights(K):
    """[64,64,5,5,5] fp32 -> DoubleRow lhsT tiles [128, NG, 2, 128] fp8.

    Row-half r of k-tile t holds the blocks for plane slot+r; column half m
    (out plane z+m) holds tap dz = plane - m from Khi or Klo per the group's
    class assignment."""
    Ks = K * SK
    Khi = Ks.astype(E4).astype(np.float32)
    Klo = (Ks - Khi).astype(E4).astype(np.float32)
    wk = np.zeros((128, NG, 2, 128), np.float32)
    for g, (dy, dx, kind, sA, sB, cA, cB) in enumerate(GROUPS):
        for t, (slot, cls) in enumerate(((sA, cA), (sB, cB))):
            if cls is None:
                continue
            Kc = Khi if cls == 0 else Klo
            for r in range(2):
                p = slot + r
                for m in range(2):
                    dz = p - m
                    if 0 <= dz < 5:
                        wk[64 * r : 64 * r + 64, g, t, 64 * m : 64 * m + 64] = Kc[
                            :, :, dz, dy, dx
                        ].T
    return np.ascontiguousarray(wk.astype(E4))


def _pack_x(x):
    """x [1,64,56,56,56] -> per-core fp8 hi/lo slabs:
    (hi [64,10,56,56], lo [64,10,56,56], hi2 [64,6,17,56], lo2 [...])."""
    xs = x[0] * SX
    xhi = xs.astype(E4)
    xlo = (xs - xhi.astype(np.float32)).astype(E4)
    slabs = []
    for c in range(N_CORES):
        z0, p2, y2 = _core_assign(c)
        slabs.append((
            np.ascontiguousarray(xhi[:, z0 : z0 + D_DRAM]),
            np.ascontiguousarray(xlo[:, z0 : z0 + D_DRAM]),
            np.ascontiguousarray(xhi[:, p2 : p2 + P2_DRAM, y2 : y2 + P2_IN_LINES]),
            np.ascontiguousarray(xlo[:, p2 : p2 + P2_DRAM, y2 : y2 + P2_IN_LINES]),
        ))
    return slabs


# ------------------------------------------------------- device program
def build_nc(n_pairs=N_PAIRS, partial=True, repeat=1):
    fp8 = mybir.dt.float8e4
    fp32 = mybir.dt.float32
    nc = bacc.Bacc("TRN2", target_bir_lowering=False, debug=False,
                   num_devices=N_CORES)
    xh_d = nc.dram_tensor("xh", [64, D_DRAM, S, S], fp8, kind="ExternalInput").ap()
    xl_d = nc.dram_tensor("xl", [64, D_DRAM, S, S], fp8, kind="ExternalInput").ap()
    xh2_d = nc.dram_tensor("xh2", [64, P2_DRAM, P2_IN_LINES, S], fp8,
                           kind="ExternalInput").ap()
    xl2_d = nc.dram_tensor("xl2", [64, P2_DRAM, P2_IN_LINES, S], fp8,
                           kind="ExternalInput").ap()
    w_d = nc.dram_tensor("w", [128, NG, 2, 128], fp8, kind="ExternalInput").ap()
    # outputs are plane-major so one DMA can write both planes of a pair:
    # SBUF partitions (z c) = plane-half * 64 + channel
    o_d = nc.dram_tensor("out", [2 * n_pairs, 64, SO, SO], fp32,
                         kind="ExternalOutput").ap()
    o2_d = nc.dram_tensor("out2", [2, 64, P2_LINES, SO], fp32,
                          kind="ExternalOutput").ap()

    DR = mybir.MatmulPerfMode.DoubleRow

    with tile.TileContext(nc) as tc:
        with (
            tc.tile_pool(name="const", bufs=1) as cpool,
            tc.tile_pool(name="outp", bufs=3) as opool,
            tc.tile_pool(name="psum", bufs=8, space="PSUM") as ppool,
        ):
            xh = cpool.tile([128, D_SB, S, S], fp8)
            xl = cpool.tile([128, D_SB, S, S], fp8)
            xh2 = cpool.tile([128, P2_SB, P2_IN_LINES, S], fp8)
            xl2 = cpool.tile([128, P2_SB, P2_IN_LINES, S], fp8)
            wt = cpool.tile([128, NG, 2, 128], fp8)
            # DMA order = first-use order, split across BOTH HWDGE queues
            # (SP = nc.sync, Activation = nc.scalar).  Lead DMAs cover only
            # what the first chunk's phase-0 groups touch (partial slab
            # slots 0..3, lines 0..10) so the first matmul fires early.
            # Dual z-shifted SBUF copies: partitions 0..63 plane j <- plane
            # j, partitions 64..127 plane j <- plane j+1.  Weight slices are
            # interleaved 2:1 ahead of the x planes: fp8 matmuls drain a
            # weight slice (3 groups) every ~260ns, faster than a 1:1
            # interleave can supply them.
            # head: weight slice 0 + partial-slab leads (slots 0..3, lines
            # 0..10 — exactly what the first chunk's phase-0 groups read)
            nc.sync.dma_start(wt[:, 0:3], w_d[:, 0:3])
            nc.sync.dma_start(xh2[:64, :4, :11], xh2_d[:, :4, :11])
            nc.sync.dma_start(xh2[64:, :4, :11], xh2_d[:, 1:5, :11])
            nc.sync.dma_start(xh2[:64, :4, 11:], xh2_d[:, :4, 11:])
            nc.sync.dma_start(xh2[64:, :4, 11:], xh2_d[:, 1:5, 11:])
            nc.sync.dma_start(xh2[:64, 4], xh2_d[:, 4])
            nc.sync.dma_start(xh2[64:, 4], xh2_d[:, 5])
            nc.sync.dma_start(xl2[:64], xl2_d[:, :P2_SB])
            nc.sync.dma_start(xl2[64:], xl2_d[:, 1 : P2_SB + 1])
            # the partial chunks (first ~11us of compute) touch every weight
            # slice; front-load half the weight stream before the main slab,
            # then interleave the rest with hi/lo plane pairs in first-use
            # order (pair-0 chunks read xh AND xl slots 0..4)
            wops = [
                lambda i=i: nc.sync.dma_start(wt[:, 3 * i : 3 * (i + 1)],
                                              w_d[:, 3 * i : 3 * (i + 1)])
                for i in range(1, NG // 3)
            ]
            for _ in range(13):
                wops.pop(0)()
            xops = []
            for j in range(D_SB):
                xops.append(lambda j=j: nc.sync.dma_start(xh[:64, j], xh_d[:, j]))
                xops.append(
                    lambda j=j: nc.sync.dma_start(xh[64:, j], xh_d[:, j + 1])
                )
                xops.append(lambda j=j: nc.sync.dma_start(xl[:64, j], xl_d[:, j]))
                xops.append(
                    lambda j=j: nc.sync.dma_start(xl[64:, j], xl_d[:, j + 1])
                )
            while wops or xops:
                if wops:
                    wops.pop(0)()
                for _ in range(4):
                    if xops:
                        xops.pop(0)()

            def do_chunk(hi, lo, z, ys, L, dst, zo):
                ps = ppool.tile([128, 9, SO], fp32)
                for g, (dy, dx, kind, sA, sB, cA, cB) in enumerate(GROUPS):
                    src = hi if kind == 0 else lo
                    rhs = src[:, z + sA : z + sB + 1 : (sB - sA),
                              ys + dy : ys + dy + L, dx : dx + SO]
                    nc.tensor.matmul(ps[:, :L, :], wt[:, g], rhs,
                                     start=(g == 0), stop=(g == NG - 1),
                                     perf_mode=DR)
                ot = opool.tile([128, 9, SO], fp32)
                nc.vector.tensor_copy(ot[:, :L], ps[:, :L])
                dst2 = dst[zo : zo + 2, :, ys : ys + L, :].rearrange(
                    "z c l w -> (z c) l w"
                )
                nc.sync.dma_start(dst2, ot[:, :L])

            for _ in range(repeat):
                # partial first: its input lands quickly, hiding the main
                # slab's DMA behind the partial block's compute
                if partial:
                    for ys, L in CHUNKS2:
                        do_chunk(xh2, xl2, 0, ys, L, o2_d, 0)
                for p in range(n_pairs):
                    chunks = CHUNKS_LAST if p == n_pairs - 1 else CHUNKS
                    for ys, L in chunks:
                        do_chunk(xh, xl, 2 * p, ys, L, o_d, 2 * p)
    nc.compile()
    return nc


# ------------------------------------------------------------ entry point
LAST_RESULTS = None
LAST_NC = None
LAST_INMAPS = None


def kernel(x, weight, w_sc0, w_sc1):
    global LAST_RESULTS, LAST_NC, LAST_INMAPS
    x = np.asarray(x, dtype=np.float32)
    K = _fold_self_connection(
        _make_kernel(np.asarray(weight, dtype=np.float32)),
        np.asarray(w_sc0, dtype=np.float32),
        np.asarray(w_sc1, dtype=np.float32),
    )
    wk = _pack_weights(K)
    slabs = _pack_x(x)

    repeat = int(os.environ.get("KERNEL_REPEAT", "1"))
    nc = build_nc(repeat=repeat)
    in_maps = [
        {"xh": slabs[c][0], "xl": slabs[c][1], "xh2": slabs[c][2],
         "xl2": slabs[c][3], "w": wk}
        for c in range(N_CORES)
    ]
    res = run_bass_kernel_spmd(nc, in_maps, core_ids=list(range(N_CORES)))
    LAST_RESULTS, LAST_NC, LAST_INMAPS = res, nc, in_maps

    full = np.zeros((1, 64, SO, SO, SO), np.float32)
    for c in range(N_CORES):
        z0, p2, y2 = _core_assign(c)
        # device outputs are plane-major [z, c, l, w]
        full[0, :, z0 : z0 + D_OUT] = res.results[c]["out"].transpose(1, 0, 2, 3)
        full[0, :, p2 : p2 + 2, y2 : y2 + P2_LINES, :] = res.results[c][
            "out2"
        ].transpose(1, 0, 2, 3)
    return full * OUT_SCALE


# revision 10
# speedup vs baseline: 1.0345x; 1.0054x over previous
"""Trainium2 Bass kernel for nn_Convolution_1176821039998.

Equivariant (e3nn-style) 3D convolution, kernel 5x5x5, 64->64 channels, on a
[1,64,56,56,56] fp32 volume, plus a per-irrep self-connection on the cropped
volume.  Strategy:

Host side (tiny, fp32):
  - Build the dense conv kernel K[o,i,dz,dy,dx] from the TP weight exactly as
    the reference does, and fold the self-connection into the center tap.
  - Perfectly balanced z-shard across 8 cores with NO redundant compute:
    core c computes 6 "main" output planes 6c..6c+5 (planes 0..47) plus a
    13-line y-block of one of the remaining 2 plane-pairs (planes 48..51,
    block chosen by core index).  The partial block's input sub-volume is
    packed by the host into a small side tensor at FIXED local coordinates,
    so all cores run the identical SPMD program (3.25 plane-pairs each).
  - The kernel halo (4 planes) is handled by overlapping shards; no
    device-to-device exchange.

Zero-tap structure: the equivariant kernel is EXACTLY zero for taps with
lattice distance >= 1.25 (the smooth_finite radial embedding vanishes):
  K[:,:,dz,dy,dx] == 0  iff  (dz-2)^2+(dy-2)^2+(dx-2)^2 >= 7.
Per (dy,dx) column with rho2=(dy-2)^2+(dx-2)^2: rho2<=2 (9 cols) keeps all
5 dz taps; rho2 in {4,5} (12 cols) keeps only dz in {1,2,3}; the 4 corner
columns are fully zero and skipped.

Device side (fp8e4m3 DoubleRow matmuls, 0.5 PE cycles per output column):
  - Operands are split hi/lo: xs=8*x -> xhi=e4m3(xs), xlo=e4m3(xs-xhi);
    Ks=32*K -> Khi, Klo likewise.  The product is computed as
    Khi*xhi + Klo*xhi + Khi*xlo (the Klo*xlo term is dropped; measured
    max-rel-err of the scheme vs fp32 reference is ~2e-3).  Host divides
    the gathered output by 256.
  - SBUF holds dual z-shifted copies (partitions 0..63 plane j, 64..127
    plane j+1) of the hi and lo volumes.  A DoubleRow matmul contracts TWO
    such k-tiles (at a constant plane stride, expressed as a strided slice)
    against a [128, 2, 128] fp8 weight pair, accumulating 4 plane-taps per
    instruction into a PSUM pair of output planes (M = 64 ch x 2 planes).
  - Per (dy,dx) column and plane-pair: rho2<=2 needs k-tile slots
    {0,0,2,2,4,4} (hi, classes hi/lo per plane) + {0,2,4} (lo volume, hi
    class) -> 5 DR matmuls; rho2 in {4,5}: slots {1,1,3,3} + {1,3} -> 3.
    81 DR matmuls per chunk (vs 51 fp16 matmuls = 2x fewer PE cycles).
"""

import os
import numpy as np
import ml_dtypes

import concourse.bass as bass
import concourse.mybir as mybir
import concourse.tile as tile
from concourse import bacc
from concourse.bass_utils import run_bass_kernel_spmd

# ---------------------------------------------------------------- constants
SIZE = 5
MUL = 16
CROP = SIZE // 2
PW0 = np.float32((1.0 / 32.0) ** 0.5)
PW1 = np.float32((3.0 / 32.0) ** 0.5)
INV_SQRT3 = np.float32(3.0 ** -0.5)

N_CORES = 8
S = 56                                 # input spatial size
SO = 52                                # output spatial size
# main shard: 3 pairs = 6 output planes at z0 = 6c (covers 0..47)
N_PAIRS = 3
D_OUT = 6
D_DRAM = 10                            # main DRAM slab planes (6c..6c+9)
D_SB = 9                               # planes per SBUF copy (lo 0..8, hi 1..9)
# partial shard: planes 48..51 split into 8 (pair, 13-line block) quarters
P2_BASE = 48
P2_LINES = 13
P2_IN_LINES = P2_LINES + 4             # 17
P2_DRAM = 6                            # partial DRAM planes (P..P+5)
P2_SB = 5                              # partial SBUF planes per copy
CHUNKS = [(0, 9), (9, 9), (18, 9), (27, 9), (36, 9), (45, 7)]  # (y0, lines)
# last pair ends with a tiny chunk so the final PSUM-evict + store DMA tail
# after the last matmul is short
CHUNKS_LAST = [(0, 9), (9, 9), (18, 9), (27, 9), (36, 9), (45, 5), (50, 2)]
CHUNKS2 = [(0, 7), (7, 6)]             # partial 13-line block

E4 = ml_dtypes.float8_e4m3fn
SX = np.float32(8.0)                   # x scale before fp8 quantization
SK = np.float32(32.0)                  # K scale before fp8 quantization
OUT_SCALE = np.float32(1.0 / (8.0 * 32.0))


def _dr_groups():
    """DoubleRow matmul groups per (chunk x plane-pair).

    Each entry: (dy, dx, kind, slotA, slotB, clsA, clsB).
    kind 0 = hi volume, 1 = lo volume.  slot = dual-copy slot relative to the
    pair base z (slot j holds planes z+j / z+j+1 in its two partition
    halves).  cls 0 = Khi, 1 = Klo, None = zero-weight padding half.
    Ordered in phases so early groups only touch low slots (DMA gating).
    """
    cols = [(dy, dx) for dy in range(5) for dx in range(5)
            if (dy - 2) ** 2 + (dx - 2) ** 2 <= 5]

    def full(dy, dx):
        return (dy - 2) ** 2 + (dx - 2) ** 2 <= 2

    phases = ([], [], [], [], [])
    for dy, dx in cols:
        if full(dy, dx):
            phases[0].append((dy, dx, 0, 0, 2, 0, 0))
            phases[1].append((dy, dx, 0, 2, 4, 1, 0))
            phases[2].append((dy, dx, 0, 0, 4, 1, 1))
            phases[3].append((dy, dx, 1, 0, 2, 0, 0))
            # the odd lo k-tile pairs with a free slot; use it for a partial
            # 4th-term (Klo*xlo, planes 2..3) correction at zero extra cost
            phases[4].append((dy, dx, 1, 2, 4, 1, 0))
        else:
            phases[0].append((dy, dx, 0, 1, 3, 0, 0))
            phases[1].append((dy, dx, 0, 1, 3, 1, 1))
            phases[3].append((dy, dx, 1, 1, 3, 0, 0))
    return [g for ph in phases for g in ph]


GROUPS = _dr_groups()
NG = len(GROUPS)  # 81


def _core_assign(c):
    """(main z0, partial pair base, partial y0) for core c."""
    return 6 * c, P2_BASE + 2 * (c // 4), P2_LINES * (c % 4)


# ------------------------------------------------------- host-side weights
def _lattice_consts():
    r = np.linspace(-1.0, 1.0, SIZE, dtype=np.float32)
    lat = np.stack(np.meshgrid(r, r, r, indexing="ij"), axis=-1)
    d = np.linalg.norm(lat.astype(np.float64), axis=-1).astype(np.float32)
    values = np.linspace(0.0, 1.0, SIZE, dtype=np.float32)
    step = values[1] - values[0]
    diff = (d[..., None] - values) / step

    def sus(t):
        return np.where(t > 0, np.exp(-1.0 / np.where(t > 0, t, 1.0)), 0.0).astype(
            np.float32
        )

    emb = np.float32(1.14136) * np.float32(np.e ** 2) * sus(diff + 1.0) * sus(1.0 - diff)
    n = lat / np.maximum(d, 1e-12)[..., None]
    sh0 = np.ones_like(d)
    sh1 = np.float32(3.0 ** 0.5) * n
    return emb.astype(np.float32), sh0, sh1.astype(np.float32)


def _make_kernel(weight):
    """[5,1024] -> conv kernel [out=64, in=64, 5,5,5] fp32 (mirrors reference)."""
    emb, sh0, sh1 = _lattice_consts()
    w = emb @ weight
    Ssp = w.shape[:3]
    blk = MUL * MUL
    w1, w2, w3, w4 = [
        w[..., i * blk : (i + 1) * blk].reshape(*Ssp, MUL, MUL) for i in range(4)
    ]
    k_ss = PW0 * w1 * sh0[..., None, None]
    k_sv = PW1 * INV_SQRT3 * np.einsum("...uw,...k->...uwk", w2, sh1)
    k_vs = PW0 * INV_SQRT3 * np.einsum("...uw,...i->...uiw", w4, sh1)
    eye3 = np.eye(3, dtype=w.dtype)
    k_vv = (
        PW1
        * INV_SQRT3
        * (w3 * sh0[..., None, None])[..., :, None, :, None]
        * eye3[None, None, None, None, :, None, :]
    )
    top = np.concatenate([k_ss, k_sv.reshape(*Ssp, MUL, 3 * MUL)], axis=-1)
    bot = np.concatenate(
        [k_vs.reshape(*Ssp, 3 * MUL, MUL), k_vv.reshape(*Ssp, 3 * MUL, 3 * MUL)],
        axis=-1,
    )
    kernel = np.concatenate([top, bot], axis=-2)  # [5,5,5,in,out]
    return np.ascontiguousarray(np.transpose(kernel, (4, 3, 0, 1, 2)))


def _fold_self_connection(K, w_sc0, w_sc1):
    """Add the cropped e3nn Linear self-connection into the center tap."""
    inv = np.float32(1.0 / MUL ** 0.5)
    sc = np.zeros((64, 64), np.float32)
    sc[:MUL, :MUL] = w_sc0.T * inv  # sc[out w, in u] = w_sc0[u, w]
    for wo in range(MUL):
        for u in range(MUL):
            for k in range(3):
                sc[MUL + 3 * wo + k, MUL + 3 * u + k] += w_sc1[u, wo] * inv
    K = K.copy()
    K[:, :, CROP, CROP, CROP] += sc
    return K


def _pack_weights(K):
    """[64,64,5,5,5] fp32 -> DoubleRow lhsT tiles [128, NG, 2, 128] fp8.

    Row-half r of k-tile t holds the blocks for plane slot+r; column half m
    (out plane z+m) holds tap dz = plane - m from Khi or Klo per the group's
    class assignment."""
    Ks = K * SK
    Khi = Ks.astype(E4).astype(np.float32)
    Klo = (Ks - Khi).astype(E4).astype(np.float32)
    wk = np.zeros((128, NG, 2, 128), np.float32)
    for g, (dy, dx, kind, sA, sB, cA, cB) in enumerate(GROUPS):
        for t, (slot, cls) in enumerate(((sA, cA), (sB, cB))):
            if cls is None:
                continue
            Kc = Khi if cls == 0 else Klo
            for r in range(2):
                p = slot + r
                for m in range(2):
                    dz = p - m
                    if 0 <= dz < 5:
                        wk[64 * r : 64 * r + 64, g, t, 64 * m : 64 * m + 64] = Kc[
                            :, :, dz, dy, dx
                        ].T
    return np.ascontiguousarray(wk.astype(E4))


def _pack_x(x):
    """x [1,64,56,56,56] -> per-core fp8 hi/lo slabs:
    (hi [64,10,56,56], lo [64,10,56,56], hi2 [64,6,17,56], lo2 [...])."""
    xs = x[0] * SX
    xhi = xs.astype(E4)
    xlo = (xs - xhi.astype(np.float32)).astype(E4)
    slabs = []
    for c in range(N_CORES):
        z0, p2, y2 = _core_assign(c)
        slabs.append((
            np.ascontiguousarray(xhi[:, z0 : z0 + D_DRAM]),
            np.ascontiguousarray(xlo[:, z0 : z0 + D_DRAM]),
            np.ascontiguousarray(xhi[:, p2 : p2 + P2_DRAM, y2 : y2 + P2_IN_LINES]),
            np.ascontiguousarray(xlo[:, p2 : p2 + P2_DRAM, y2 : y2 + P2_IN_LINES]),
        ))
    return slabs


# ------------------------------------------------------- device program
def build_nc(n_pairs=N_PAIRS, partial=True, repeat=1):
    fp8 = mybir.dt.float8e4
    fp32 = mybir.dt.float32
    nc = bacc.Bacc("TRN2", target_bir_lowering=False, debug=False,
                   num_devices=N_CORES)
    xh_d = nc.dram_tensor("xh", [64, D_DRAM, S, S], fp8, kind="ExternalInput").ap()
    xl_d = nc.dram_tensor("xl", [64, D_DRAM, S, S], fp8, kind="ExternalInput").ap()
    xh2_d = nc.dram_tensor("xh2", [64, P2_DRAM, P2_IN_LINES, S], fp8,
                           kind="ExternalInput").ap()
    xl2_d = nc.dram_tensor("xl2", [64, P2_DRAM, P2_IN_LINES, S], fp8,
                           kind="ExternalInput").ap()
    w_d = nc.dram_tensor("w", [128, NG, 2, 128], fp8, kind="ExternalInput").ap()
    # outputs are plane-major so one DMA can write both planes of a pair:
    # SBUF partitions (z c) = plane-half * 64 + channel
    o_d = nc.dram_tensor("out", [2 * n_pairs, 64, SO, SO], fp32,
                         kind="ExternalOutput").ap()
    o2_d = nc.dram_tensor("out2", [2, 64, P2_LINES, SO], fp32,
                          kind="ExternalOutput").ap()

    DR = mybir.MatmulPerfMode.DoubleRow

    with tile.TileContext(nc) as tc:
        with (
            tc.tile_pool(name="const", bufs=1) as cpool,
            tc.tile_pool(name="outp", bufs=3) as opool,
            tc.tile_pool(name="psum", bufs=8, space="PSUM") as ppool,
        ):
            xh = cpool.tile([128, D_SB, S, S], fp8)
            xl = cpool.tile([128, D_SB, S, S], fp8)
            xh2 = cpool.tile([128, P2_SB, P2_IN_LINES, S], fp8)
            xl2 = cpool.tile([128, P2_SB, P2_IN_LINES, S], fp8)
            wt = cpool.tile([128, NG, 2, 128], fp8)
            # DMA order = first-use order, split across BOTH HWDGE queues
            # (SP = nc.sync, Activation = nc.scalar).  Lead DMAs cover only
            # what the first chunk's phase-0 groups touch (partial slab
            # slots 0..3, lines 0..10) so the first matmul fires early.
            # Dual z-shifted SBUF copies: partitions 0..63 plane j <- plane
            # j, partitions 64..127 plane j <- plane j+1.  Weight slices are
            # interleaved 2:1 ahead of the x planes: fp8 matmuls drain a
            # weight slice (3 groups) every ~260ns, faster than a 1:1
            # interleave can supply them.
            # head: weight slice 0 + partial-slab leads (slots 0..3, lines
            # 0..10 — exactly what the first chunk's phase-0 groups read)
            nc.sync.dma_start(wt[:, 0:3], w_d[:, 0:3])
            nc.sync.dma_start(xh2[:64, :4, :11], xh2_d[:, :4, :11])
            nc.sync.dma_start(xh2[64:, :4, :11], xh2_d[:, 1:5, :11])
            nc.sync.dma_start(xh2[:64, :4, 11:], xh2_d[:, :4, 11:])
            nc.sync.dma_start(xh2[64:, :4, 11:], xh2_d[:, 1:5, 11:])
            nc.sync.dma_start(xh2[:64, 4], xh2_d[:, 4])
            nc.sync.dma_start(xh2[64:, 4], xh2_d[:, 5])
            # remaining stream in first-use order: early weight slices (the
            # first chunk consumes a slice every ~230ns), xl2 before the
            # first chunk's lo phase, then xh planes 0..5 / xl planes 0..5
            # for pair 0, then the leftovers 1:1 with the last weight slices
            wops = [
                lambda i=i: nc.sync.dma_start(wt[:, 3 * i : 3 * (i + 1)],
                                              w_d[:, 3 * i : 3 * (i + 1)])
                for i in range(1, NG // 3)
            ]
            for _ in range(7):
                wops.pop(0)()
            nc.sync.dma_start(xl2[:64], xl2_d[:, :P2_SB])
            nc.sync.dma_start(xl2[64:], xl2_d[:, 1 : P2_SB + 1])
            for _ in range(9):
                wops.pop(0)()
            for j in range(5):
                nc.sync.dma_start(xh[:64, j], xh_d[:, j])
                nc.sync.dma_start(xh[64:, j], xh_d[:, j + 1])
            for j in range(5):
                nc.sync.dma_start(xl[:64, j], xl_d[:, j])
                nc.sync.dma_start(xl[64:, j], xl_d[:, j + 1])
            xops = []
            for j in range(5, D_SB):
                xops.append(lambda j=j: nc.sync.dma_start(xh[:64, j], xh_d[:, j]))
                xops.append(
                    lambda j=j: nc.sync.dma_start(xh[64:, j], xh_d[:, j + 1])
                )
                xops.append(lambda j=j: nc.sync.dma_start(xl[:64, j], xl_d[:, j]))
                xops.append(
                    lambda j=j: nc.sync.dma_start(xl[64:, j], xl_d[:, j + 1])
                )
            while wops or xops:
                if wops:
                    wops.pop(0)()
                if xops:
                    xops.pop(0)()

            def do_chunk(hi, lo, z, ys, L, dst, zo):
                ps = ppool.tile([128, 9, SO], fp32)
                for g, (dy, dx, kind, sA, sB, cA, cB) in enumerate(GROUPS):
                    src = hi if kind == 0 else lo
                    rhs = src[:, z + sA : z + sB + 1 : (sB - sA),
                              ys + dy : ys + dy + L, dx : dx + SO]
                    nc.tensor.matmul(ps[:, :L, :], wt[:, g], rhs,
                                     start=(g == 0), stop=(g == NG - 1),
                                     perf_mode=DR)
                ot = opool.tile([128, 9, SO], fp32)
                nc.vector.tensor_copy(ot[:, :L], ps[:, :L])
                dst2 = dst[zo : zo + 2, :, ys : ys + L, :].rearrange(
                    "z c l w -> (z c) l w"
                )
                nc.sync.dma_start(dst2, ot[:, :L])

            for _ in range(repeat):
                # partial first: its input lands quickly, hiding the main
                # slab's DMA behind the partial block's compute
                if partial:
                    for ys, L in CHUNKS2:
                        do_chunk(xh2, xl2, 0, ys, L, o2_d, 0)
                for p in range(n_pairs):
                    chunks = CHUNKS_LAST if p == n_pairs - 1 else CHUNKS
                    for ys, L in chunks:
                        do_chunk(xh, xl, 2 * p, ys, L, o_d, 2 * p)
    nc.compile()
    return nc


# ------------------------------------------------------------ entry point
LAST_RESULTS = None
LAST_NC = None
LAST_INMAPS = None


def kernel(x, weight, w_sc0, w_sc1):
    global LAST_RESULTS, LAST_NC, LAST_INMAPS
    x = np.asarray(x, dtype=np.float32)
    K = _fold_self_connection(
        _make_kernel(np.asarray(weight, dtype=np.float32)),
        np.asarray(w_sc0, dtype=np.float32),
        np.asarray(w_sc1, dtype=np.float32),
    )
    wk = _pack_weights(K)
    slabs = _pack_x(x)

    repeat = int(os.environ.get("KERNEL_REPEAT", "1"))
    nc = build_nc(repeat=repeat)
    in_maps = [
        {"xh": slabs[c][0], "xl": slabs[c][1], "xh2": slabs[c][2],
         "xl2": slabs[c][3], "w": wk}
        for c in range(N_CORES)
    ]
    res = run_bass_kernel_spmd(nc, in_maps, core_ids=list(range(N_CORES)))
    LAST_RESULTS, LAST_NC, LAST_INMAPS = res, nc, in_maps

    full = np.zeros((1, 64, SO, SO, SO), np.float32)
    for c in range(N_CORES):
        z0, p2, y2 = _core_assign(c)
        # device outputs are plane-major [z, c, l, w]
        full[0, :, z0 : z0 + D_OUT] = res.results[c]["out"].transpose(1, 0, 2, 3)
        full[0, :, p2 : p2 + 2, y2 : y2 + P2_LINES, :] = res.results[c][
            "out2"
        ].transpose(1, 0, 2, 3)
    return full * OUT_SCALE


# revision 11
# speedup vs baseline: 1.0492x; 1.0142x over previous
"""Trainium2 Bass kernel for nn_Convolution_1176821039998.

Equivariant (e3nn-style) 3D convolution, kernel 5x5x5, 64->64 channels, on a
[1,64,56,56,56] fp32 volume, plus a per-irrep self-connection on the cropped
volume.  Strategy:

Host side (tiny, fp32):
  - Build the dense conv kernel K[o,i,dz,dy,dx] from the TP weight exactly as
    the reference does, and fold the self-connection into the center tap.
  - Perfectly balanced z-shard across 8 cores with NO redundant compute:
    core c computes 6 "main" output planes 6c..6c+5 (planes 0..47) plus a
    13-line y-block of one of the remaining 2 plane-pairs (planes 48..51,
    block chosen by core index).  The partial block's input sub-volume is
    packed by the host into a small side tensor at FIXED local coordinates,
    so all cores run the identical SPMD program (3.25 plane-pairs each).
  - The kernel halo (4 planes) is handled by overlapping shards; no
    device-to-device exchange.

Zero-tap structure: the equivariant kernel is EXACTLY zero for taps with
lattice distance >= 1.25 (the smooth_finite radial embedding vanishes):
  K[:,:,dz,dy,dx] == 0  iff  (dz-2)^2+(dy-2)^2+(dx-2)^2 >= 7.
Per (dy,dx) column with rho2=(dy-2)^2+(dx-2)^2: rho2<=2 (9 cols) keeps all
5 dz taps; rho2 in {4,5} (12 cols) keeps only dz in {1,2,3}; the 4 corner
columns are fully zero and skipped.

Device side (fp8e4m3 DoubleRow matmuls, 0.5 PE cycles per output column):
  - Operands are split hi/lo: xs=8*x -> xhi=e4m3(xs), xlo=e4m3(xs-xhi);
    Ks=32*K -> Khi, Klo likewise.  The product is computed as
    Khi*xhi + Klo*xhi + Khi*xlo (the Klo*xlo term is dropped; measured
    max-rel-err of the scheme vs fp32 reference is ~2e-3).  Host divides
    the gathered output by 256.
  - SBUF holds dual z-shifted copies (partitions 0..63 plane j, 64..127
    plane j+1) of the hi and lo volumes.  A DoubleRow matmul contracts TWO
    such k-tiles (at a constant plane stride, expressed as a strided slice)
    against a [128, 2, 128] fp8 weight pair, accumulating 4 plane-taps per
    instruction into a PSUM pair of output planes (M = 64 ch x 2 planes).
  - Per (dy,dx) column and plane-pair: rho2<=2 needs k-tile slots
    {0,0,2,2,4,4} (hi, classes hi/lo per plane) + {0,2,4} (lo volume, hi
    class) -> 5 DR matmuls; rho2 in {4,5}: slots {1,1,3,3} + {1,3} -> 3.
    81 DR matmuls per chunk (vs 51 fp16 matmuls = 2x fewer PE cycles).
"""

import os
import numpy as np
import ml_dtypes

import concourse.bass as bass
import concourse.mybir as mybir
import concourse.tile as tile
from concourse import bacc
from concourse.bass_utils import run_bass_kernel_spmd

# ---------------------------------------------------------------- constants
SIZE = 5
MUL = 16
CROP = SIZE // 2
PW0 = np.float32((1.0 / 32.0) ** 0.5)
PW1 = np.float32((3.0 / 32.0) ** 0.5)
INV_SQRT3 = np.float32(3.0 ** -0.5)

N_CORES = 8
S = 56                                 # input spatial size
SO = 52                                # output spatial size
# main shard: 3 pairs = 6 output planes at z0 = 6c (covers 0..47)
N_PAIRS = 3
D_OUT = 6
D_DRAM = 10                            # main DRAM slab planes (6c..6c+9)
D_SB = 9                               # planes per SBUF copy (lo 0..8, hi 1..9)
# partial shard: planes 48..51 split into 8 (pair, 13-line block) quarters
P2_BASE = 48
P2_LINES = 13
P2_IN_LINES = P2_LINES + 4             # 17
P2_DRAM = 6                            # partial DRAM planes (P..P+5)
P2_SB = 5                              # partial SBUF planes per copy
CHUNKS = [(0, 9), (9, 9), (18, 9), (27, 9), (36, 9), (45, 7)]  # (y0, lines)
# last pair ends with a tiny chunk so the final PSUM-evict + store DMA tail
# after the last matmul is short
CHUNKS_LAST = [(0, 9), (9, 9), (18, 9), (27, 9), (36, 9), (45, 5), (50, 2)]
CHUNKS2 = [(0, 7), (7, 6)]             # partial 13-line block

E4 = ml_dtypes.float8_e4m3fn
SX = np.float32(8.0)                   # x scale before fp8 quantization
SK = np.float32(32.0)                  # K scale before fp8 quantization
OUT_SCALE = np.float32(1.0 / (8.0 * 32.0))


def _dr_groups():
    """DoubleRow matmul groups per (chunk x plane-pair).

    Each entry: (dy, dx, kind, slotA, slotB, clsA, clsB).
    kind 0 = hi volume, 1 = lo volume.  slot = dual-copy slot relative to the
    pair base z (slot j holds planes z+j / z+j+1 in its two partition
    halves).  cls 0 = Khi, 1 = Klo, None = zero-weight padding half.
    Ordered in phases so early groups only touch low slots (DMA gating).
    """
    cols = [(dy, dx) for dy in range(5) for dx in range(5)
            if (dy - 2) ** 2 + (dx - 2) ** 2 <= 5]

    def full(dy, dx):
        return (dy - 2) ** 2 + (dx - 2) ** 2 <= 2

    phases = ([], [], [], [], [])
    for dy, dx in cols:
        if full(dy, dx):
            phases[0].append((dy, dx, 0, 0, 2, 0, 0))
            phases[1].append((dy, dx, 0, 2, 4, 1, 0))
            phases[2].append((dy, dx, 0, 0, 4, 1, 1))
            phases[3].append((dy, dx, 1, 0, 2, 0, 0))
            # the odd lo k-tile pairs with a free slot; use it for a partial
            # 4th-term (Klo*xlo, planes 2..3) correction at zero extra cost
            phases[4].append((dy, dx, 1, 2, 4, 1, 0))
        else:
            phases[0].append((dy, dx, 0, 1, 3, 0, 0))
            phases[1].append((dy, dx, 0, 1, 3, 1, 1))
            phases[3].append((dy, dx, 1, 1, 3, 0, 0))
    return [g for ph in phases for g in ph]


GROUPS = _dr_groups()
NG = len(GROUPS)  # 81


def _core_assign(c):
    """(main z0, partial pair base, partial y0) for core c."""
    return 6 * c, P2_BASE + 2 * (c // 4), P2_LINES * (c % 4)


# ------------------------------------------------------- host-side weights
def _lattice_consts():
    r = np.linspace(-1.0, 1.0, SIZE, dtype=np.float32)
    lat = np.stack(np.meshgrid(r, r, r, indexing="ij"), axis=-1)
    d = np.linalg.norm(lat.astype(np.float64), axis=-1).astype(np.float32)
    values = np.linspace(0.0, 1.0, SIZE, dtype=np.float32)
    step = values[1] - values[0]
    diff = (d[..., None] - values) / step

    def sus(t):
        return np.where(t > 0, np.exp(-1.0 / np.where(t > 0, t, 1.0)), 0.0).astype(
            np.float32
        )

    emb = np.float32(1.14136) * np.float32(np.e ** 2) * sus(diff + 1.0) * sus(1.0 - diff)
    n = lat / np.maximum(d, 1e-12)[..., None]
    sh0 = np.ones_like(d)
    sh1 = np.float32(3.0 ** 0.5) * n
    return emb.astype(np.float32), sh0, sh1.astype(np.float32)


def _make_kernel(weight):
    """[5,1024] -> conv kernel [out=64, in=64, 5,5,5] fp32 (mirrors reference)."""
    emb, sh0, sh1 = _lattice_consts()
    w = emb @ weight
    Ssp = w.shape[:3]
    blk = MUL * MUL
    w1, w2, w3, w4 = [
        w[..., i * blk : (i + 1) * blk].reshape(*Ssp, MUL, MUL) for i in range(4)
    ]
    k_ss = PW0 * w1 * sh0[..., None, None]
    k_sv = PW1 * INV_SQRT3 * np.einsum("...uw,...k->...uwk", w2, sh1)
    k_vs = PW0 * INV_SQRT3 * np.einsum("...uw,...i->...uiw", w4, sh1)
    eye3 = np.eye(3, dtype=w.dtype)
    k_vv = (
        PW1
        * INV_SQRT3
        * (w3 * sh0[..., None, None])[..., :, None, :, None]
        * eye3[None, None, None, None, :, None, :]
    )
    top = np.concatenate([k_ss, k_sv.reshape(*Ssp, MUL, 3 * MUL)], axis=-1)
    bot = np.concatenate(
        [k_vs.reshape(*Ssp, 3 * MUL, MUL), k_vv.reshape(*Ssp, 3 * MUL, 3 * MUL)],
        axis=-1,
    )
    kernel = np.concatenate([top, bot], axis=-2)  # [5,5,5,in,out]
    return np.ascontiguousarray(np.transpose(kernel, (4, 3, 0, 1, 2)))


def _fold_self_connection(K, w_sc0, w_sc1):
    """Add the cropped e3nn Linear self-connection into the center tap."""
    inv = np.float32(1.0 / MUL ** 0.5)
    sc = np.zeros((64, 64), np.float32)
    sc[:MUL, :MUL] = w_sc0.T * inv  # sc[out w, in u] = w_sc0[u, w]
    for wo in range(MUL):
        for u in range(MUL):
            for k in range(3):
                sc[MUL + 3 * wo + k, MUL + 3 * u + k] += w_sc1[u, wo] * inv
    K = K.copy()
    K[:, :, CROP, CROP, CROP] += sc
    return K


def _pack_weights(K):
    """[64,64,5,5,5] fp32 -> DoubleRow lhsT tiles [128, NG, 2, 128] fp8.

    Row-half r of k-tile t holds the blocks for plane slot+r; column half m
    (out plane z+m) holds tap dz = plane - m from Khi or Klo per the group's
    class assignment."""
    Ks = K * SK
    Khi = Ks.astype(E4).astype(np.float32)
    Klo = (Ks - Khi).astype(E4).astype(np.float32)
    wk = np.zeros((128, NG, 2, 128), np.float32)
    for g, (dy, dx, kind, sA, sB, cA, cB) in enumerate(GROUPS):
        for t, (slot, cls) in enumerate(((sA, cA), (sB, cB))):
            if cls is None:
                continue
            Kc = Khi if cls == 0 else Klo
            for r in range(2):
                p = slot + r
                for m in range(2):
                    dz = p - m
                    if 0 <= dz < 5:
                        wk[64 * r : 64 * r + 64, g, t, 64 * m : 64 * m + 64] = Kc[
                            :, :, dz, dy, dx
                        ].T
    return np.ascontiguousarray(wk.astype(E4))


def _pack_x(x):
    """x [1,64,56,56,56] -> per-core fp8 hi/lo slabs:
    (hi [64,10,56,56], lo [64,10,56,56], hi2 [64,6,17,56], lo2 [...])."""
    xs = x[0] * SX
    xhi = xs.astype(E4)
    xlo = (xs - xhi.astype(np.float32)).astype(E4)
    slabs = []
    for c in range(N_CORES):
        z0, p2, y2 = _core_assign(c)
        slabs.append((
            np.ascontiguousarray(xhi[:, z0 : z0 + D_DRAM]),
            np.ascontiguousarray(xlo[:, z0 : z0 + D_DRAM]),
            np.ascontiguousarray(xhi[:, p2 : p2 + P2_DRAM, y2 : y2 + P2_IN_LINES]),
            np.ascontiguousarray(xlo[:, p2 : p2 + P2_DRAM, y2 : y2 + P2_IN_LINES]),
        ))
    return slabs


# ------------------------------------------------------- device program
def build_nc(n_pairs=N_PAIRS, partial=True, repeat=1):
    fp8 = mybir.dt.float8e4
    fp32 = mybir.dt.float32
    nc = bacc.Bacc("TRN2", target_bir_lowering=False, debug=False,
                   num_devices=N_CORES)
    xh_d = nc.dram_tensor("xh", [64, D_DRAM, S, S], fp8, kind="ExternalInput").ap()
    xl_d = nc.dram_tensor("xl", [64, D_DRAM, S, S], fp8, kind="ExternalInput").ap()
    xh2_d = nc.dram_tensor("xh2", [64, P2_DRAM, P2_IN_LINES, S], fp8,
                           kind="ExternalInput").ap()
    xl2_d = nc.dram_tensor("xl2", [64, P2_DRAM, P2_IN_LINES, S], fp8,
                           kind="ExternalInput").ap()
    w_d = nc.dram_tensor("w", [128, NG, 2, 128], fp8, kind="ExternalInput").ap()
    # outputs are plane-major so one DMA can write both planes of a pair:
    # SBUF partitions (z c) = plane-half * 64 + channel
    o_d = nc.dram_tensor("out", [2 * n_pairs, 64, SO, SO], fp32,
                         kind="ExternalOutput").ap()
    o2_d = nc.dram_tensor("out2", [2, 64, P2_LINES, SO], fp32,
                          kind="ExternalOutput").ap()

    DR = mybir.MatmulPerfMode.DoubleRow

    with tile.TileContext(nc) as tc:
        with (
            tc.tile_pool(name="const", bufs=1) as cpool,
            tc.tile_pool(name="outp", bufs=3) as opool,
            tc.tile_pool(name="psum", bufs=8, space="PSUM") as ppool,
        ):
            xh = cpool.tile([128, D_SB, S, S], fp8)
            xl = cpool.tile([128, D_SB, S, S], fp8)
            xh2 = cpool.tile([128, P2_SB, P2_IN_LINES, S], fp8)
            xl2 = cpool.tile([128, P2_SB, P2_IN_LINES, S], fp8)
            wt = cpool.tile([128, NG, 2, 128], fp8)
            # DMA order = first-use order, split across BOTH HWDGE queues
            # (SP = nc.sync, Activation = nc.scalar).  Lead DMAs cover only
            # what the first chunk's phase-0 groups touch (partial slab
            # slots 0..3, lines 0..10) so the first matmul fires early.
            # Dual z-shifted SBUF copies: partitions 0..63 plane j <- plane
            # j, partitions 64..127 plane j <- plane j+1.  Weight slices are
            # interleaved 2:1 ahead of the x planes: fp8 matmuls drain a
            # weight slice (3 groups) every ~260ns, faster than a 1:1
            # interleave can supply them.
            # head: weight slice 0 + partial-slab leads (slots 0..3, lines
            # 0..10 — exactly what the first chunk's phase-0 groups read)
            nc.sync.dma_start(wt[:, 0:3], w_d[:, 0:3])
            nc.sync.dma_start(xh2[:64], xh2_d[:, :P2_SB])
            nc.sync.dma_start(xh2[64:], xh2_d[:, 1 : P2_SB + 1])
            # remaining stream in first-use order: early weight slices (the
            # first chunk consumes a slice every ~230ns), xl2 before the
            # first chunk's lo phase, then xh planes 0..5 / xl planes 0..5
            # for pair 0, then the leftovers 1:1 with the last weight slices
            wops = [
                lambda i=i: nc.sync.dma_start(wt[:, 3 * i : 3 * (i + 1)],
                                              w_d[:, 3 * i : 3 * (i + 1)])
                for i in range(1, NG // 3)
            ]
            for _ in range(7):
                wops.pop(0)()
            nc.sync.dma_start(xl2[:64], xl2_d[:, :P2_SB])
            nc.sync.dma_start(xl2[64:], xl2_d[:, 1 : P2_SB + 1])
            for _ in range(9):
                wops.pop(0)()
            for j in range(5):
                nc.sync.dma_start(xh[:64, j], xh_d[:, j])
                nc.sync.dma_start(xh[64:, j], xh_d[:, j + 1])
            for j in range(5):
                nc.sync.dma_start(xl[:64, j], xl_d[:, j])
                nc.sync.dma_start(xl[64:, j], xl_d[:, j + 1])
            xops = []
            for j in range(5, D_SB):
                xops.append(lambda j=j: nc.sync.dma_start(xh[:64, j], xh_d[:, j]))
                xops.append(
                    lambda j=j: nc.sync.dma_start(xh[64:, j], xh_d[:, j + 1])
                )
                xops.append(lambda j=j: nc.sync.dma_start(xl[:64, j], xl_d[:, j]))
                xops.append(
                    lambda j=j: nc.sync.dma_start(xl[64:, j], xl_d[:, j + 1])
                )
            while wops or xops:
                if wops:
                    wops.pop(0)()
                if xops:
                    xops.pop(0)()

            def do_chunk(hi, lo, z, ys, L, dst, zo):
                ps = ppool.tile([128, 9, SO], fp32)
                for g, (dy, dx, kind, sA, sB, cA, cB) in enumerate(GROUPS):
                    src = hi if kind == 0 else lo
                    rhs = src[:, z + sA : z + sB + 1 : (sB - sA),
                              ys + dy : ys + dy + L, dx : dx + SO]
                    nc.tensor.matmul(ps[:, :L, :], wt[:, g], rhs,
                                     start=(g == 0), stop=(g == NG - 1),
                                     perf_mode=DR)
                ot = opool.tile([128, 9, SO], fp32)
                nc.vector.tensor_copy(ot[:, :L], ps[:, :L])
                dst2 = dst[zo : zo + 2, :, ys : ys + L, :].rearrange(
                    "z c l w -> (z c) l w"
                )
                nc.sync.dma_start(dst2, ot[:, :L])

            for _ in range(repeat):
                # partial first: its input lands quickly, hiding the main
                # slab's DMA behind the partial block's compute
                if partial:
                    for ys, L in CHUNKS2:
                        do_chunk(xh2, xl2, 0, ys, L, o2_d, 0)
                for p in range(n_pairs):
                    chunks = CHUNKS_LAST if p == n_pairs - 1 else CHUNKS
                    for ys, L in chunks:
                        do_chunk(xh, xl, 2 * p, ys, L, o_d, 2 * p)
    nc.compile()
    return nc


# ------------------------------------------------------------ entry point
LAST_RESULTS = None
LAST_NC = None
LAST_INMAPS = None


def kernel(x, weight, w_sc0, w_sc1):
    global LAST_RESULTS, LAST_NC, LAST_INMAPS
    x = np.asarray(x, dtype=np.float32)
    K = _fold_self_connection(
        _make_kernel(np.asarray(weight, dtype=np.float32)),
        np.asarray(w_sc0, dtype=np.float32),
        np.asarray(w_sc1, dtype=np.float32),
    )
    wk = _pack_weights(K)
    slabs = _pack_x(x)

    repeat = int(os.environ.get("KERNEL_REPEAT", "1"))
    nc = build_nc(repeat=repeat)
    in_maps = [
        {"xh": slabs[c][0], "xl": slabs[c][1], "xh2": slabs[c][2],
         "xl2": slabs[c][3], "w": wk}
        for c in range(N_CORES)
    ]
    res = run_bass_kernel_spmd(nc, in_maps, core_ids=list(range(N_CORES)))
    LAST_RESULTS, LAST_NC, LAST_INMAPS = res, nc, in_maps

    full = np.zeros((1, 64, SO, SO, SO), np.float32)
    for c in range(N_CORES):
        z0, p2, y2 = _core_assign(c)
        # device outputs are plane-major [z, c, l, w]
        full[0, :, z0 : z0 + D_OUT] = res.results[c]["out"].transpose(1, 0, 2, 3)
        full[0, :, p2 : p2 + 2, y2 : y2 + P2_LINES, :] = res.results[c][
            "out2"
        ].transpose(1, 0, 2, 3)
    return full * OUT_SCALE


# revision 12
# speedup vs baseline: 1.0850x; 1.0341x over previous
"""Trainium2 Bass kernel for nn_Convolution_1176821039998.

Equivariant (e3nn-style) 3D convolution, kernel 5x5x5, 64->64 channels, on a
[1,64,56,56,56] fp32 volume, plus a per-irrep self-connection on the cropped
volume.  Strategy:

Host side (tiny, fp32):
  - Build the dense conv kernel K[o,i,dz,dy,dx] from the TP weight exactly as
    the reference does, and fold the self-connection into the center tap.
  - Perfectly balanced z-shard across 8 cores with NO redundant compute:
    core c computes 6 "main" output planes 6c..6c+5 (planes 0..47) plus a
    13-line y-block of one of the remaining 2 plane-pairs (planes 48..51,
    block chosen by core index).  The partial block's input sub-volume is
    packed by the host into a small side tensor at FIXED local coordinates,
    so all cores run the identical SPMD program (3.25 plane-pairs each).
  - The kernel halo (4 planes) is handled by overlapping shards; no
    device-to-device exchange.

Zero-tap structure: the equivariant kernel is EXACTLY zero for taps with
lattice distance >= 1.25 (the smooth_finite radial embedding vanishes):
  K[:,:,dz,dy,dx] == 0  iff  (dz-2)^2+(dy-2)^2+(dx-2)^2 >= 7.
Per (dy,dx) column with rho2=(dy-2)^2+(dx-2)^2: rho2<=2 (9 cols) keeps all
5 dz taps; rho2 in {4,5} (12 cols) keeps only dz in {1,2,3}; the 4 corner
columns are fully zero and skipped.

Device side (fp8e4m3 DoubleRow matmuls, 0.5 PE cycles per output column):
  - Operands are split hi/lo: xs=8*x -> xhi=e4m3(xs), xlo=e4m3(xs-xhi);
    Ks=32*K -> Khi, Klo likewise.  The product is computed as
    Khi*xhi + Klo*xhi + Khi*xlo (the Klo*xlo term is dropped; measured
    max-rel-err of the scheme vs fp32 reference is ~2e-3).  Host divides
    the gathered output by 256.
  - SBUF holds dual z-shifted copies (partitions 0..63 plane j, 64..127
    plane j+1) of the hi and lo volumes.  A DoubleRow matmul contracts TWO
    such k-tiles (at a constant plane stride, expressed as a strided slice)
    against a [128, 2, 128] fp8 weight pair, accumulating 4 plane-taps per
    instruction into a PSUM pair of output planes (M = 64 ch x 2 planes).
  - Per (dy,dx) column and plane-pair: rho2<=2 needs k-tile slots
    {0,0,2,2,4,4} (hi, classes hi/lo per plane) + {0,2,4} (lo volume, hi
    class) -> 5 DR matmuls; rho2 in {4,5}: slots {1,1,3,3} + {1,3} -> 3.
    81 DR matmuls per chunk (vs 51 fp16 matmuls = 2x fewer PE cycles).
"""

import os
import numpy as np
import ml_dtypes

import concourse.bass as bass
import concourse.mybir as mybir
import concourse.tile as tile
from concourse import bacc
from concourse.bass_utils import run_bass_kernel_spmd

# ---------------------------------------------------------------- constants
SIZE = 5
MUL = 16
CROP = SIZE // 2
PW0 = np.float32((1.0 / 32.0) ** 0.5)
PW1 = np.float32((3.0 / 32.0) ** 0.5)
INV_SQRT3 = np.float32(3.0 ** -0.5)

N_CORES = 8
S = 56                                 # input spatial size
SO = 52                                # output spatial size
# main shard: 3 pairs = 6 output planes at z0 = 6c (covers 0..47)
N_PAIRS = 3
D_OUT = 6
D_DRAM = 10                            # main DRAM slab planes (6c..6c+9)
D_SB = 9                               # planes per SBUF copy (lo 0..8, hi 1..9)
# partial shard: planes 48..51 split into 8 (pair, 13-line block) quarters
P2_BASE = 48
P2_LINES = 13
P2_IN_LINES = P2_LINES + 4             # 17
P2_DRAM = 6                            # partial DRAM planes (P..P+5)
P2_SB = 5                              # partial SBUF planes per copy
CHUNKS = [(0, 9), (9, 9), (18, 9), (27, 9), (36, 9), (45, 7)]  # (y0, lines)
# last pair ends with a tiny chunk so the final PSUM-evict + store DMA tail
# after the last matmul is short
CHUNKS_LAST = [(0, 9), (9, 9), (18, 9), (27, 9), (36, 9), (45, 5), (50, 2)]
CHUNKS2 = [(0, 7), (7, 6)]             # partial 13-line block

E4 = ml_dtypes.float8_e4m3fn
SX = np.float32(8.0)                   # x scale before fp8 quantization
SK = np.float32(32.0)                  # K scale before fp8 quantization
OUT_SCALE = np.float32(1.0 / (8.0 * 32.0))


def _dr_groups():
    """DoubleRow matmul groups per (chunk x plane-pair).

    Each entry: (dy, dx, kind, slotA, slotB, clsA, clsB).
    kind 0 = hi volume, 1 = lo volume.  slot = dual-copy slot relative to the
    pair base z (slot j holds planes z+j / z+j+1 in its two partition
    halves).  cls 0 = Khi, 1 = Klo, None = zero-weight padding half.
    Ordered in phases so early groups only touch low slots (DMA gating).
    """
    cols = [(dy, dx) for dy in range(5) for dx in range(5)
            if (dy - 2) ** 2 + (dx - 2) ** 2 <= 5]

    def full(dy, dx):
        return (dy - 2) ** 2 + (dx - 2) ** 2 <= 2

    phases = ([], [], [], [], [])
    for dy, dx in cols:
        if full(dy, dx):
            phases[0].append((dy, dx, 0, 0, 2, 0, 0))
            phases[1].append((dy, dx, 0, 2, 4, 1, 0))
            phases[2].append((dy, dx, 0, 0, 4, 1, 1))
            phases[3].append((dy, dx, 1, 0, 2, 0, 0))
            # the odd lo k-tile pairs with a free slot; use it for a partial
            # 4th-term (Klo*xlo, planes 2..3) correction at zero extra cost
            phases[4].append((dy, dx, 1, 2, 4, 1, 0))
        else:
            phases[0].append((dy, dx, 0, 1, 3, 0, 0))
            phases[1].append((dy, dx, 0, 1, 3, 1, 1))
            phases[3].append((dy, dx, 1, 1, 3, 0, 0))
    return [g for ph in phases for g in ph]


GROUPS = _dr_groups()
NG = len(GROUPS)  # 81


def _core_assign(c):
    """(main z0, partial pair base, partial y0) for core c."""
    return 6 * c, P2_BASE + 2 * (c // 4), P2_LINES * (c % 4)


# ------------------------------------------------------- host-side weights
def _lattice_consts():
    r = np.linspace(-1.0, 1.0, SIZE, dtype=np.float32)
    lat = np.stack(np.meshgrid(r, r, r, indexing="ij"), axis=-1)
    d = np.linalg.norm(lat.astype(np.float64), axis=-1).astype(np.float32)
    values = np.linspace(0.0, 1.0, SIZE, dtype=np.float32)
    step = values[1] - values[0]
    diff = (d[..., None] - values) / step

    def sus(t):
        return np.where(t > 0, np.exp(-1.0 / np.where(t > 0, t, 1.0)), 0.0).astype(
            np.float32
        )

    emb = np.float32(1.14136) * np.float32(np.e ** 2) * sus(diff + 1.0) * sus(1.0 - diff)
    n = lat / np.maximum(d, 1e-12)[..., None]
    sh0 = np.ones_like(d)
    sh1 = np.float32(3.0 ** 0.5) * n
    return emb.astype(np.float32), sh0, sh1.astype(np.float32)


def _make_kernel(weight):
    """[5,1024] -> conv kernel [out=64, in=64, 5,5,5] fp32 (mirrors reference)."""
    emb, sh0, sh1 = _lattice_consts()
    w = emb @ weight
    Ssp = w.shape[:3]
    blk = MUL * MUL
    w1, w2, w3, w4 = [
        w[..., i * blk : (i + 1) * blk].reshape(*Ssp, MUL, MUL) for i in range(4)
    ]
    k_ss = PW0 * w1 * sh0[..., None, None]
    k_sv = PW1 * INV_SQRT3 * np.einsum("...uw,...k->...uwk", w2, sh1)
    k_vs = PW0 * INV_SQRT3 * np.einsum("...uw,...i->...uiw", w4, sh1)
    eye3 = np.eye(3, dtype=w.dtype)
    k_vv = (
        PW1
        * INV_SQRT3
        * (w3 * sh0[..., None, None])[..., :, None, :, None]
        * eye3[None, None, None, None, :, None, :]
    )
    top = np.concatenate([k_ss, k_sv.reshape(*Ssp, MUL, 3 * MUL)], axis=-1)
    bot = np.concatenate(
        [k_vs.reshape(*Ssp, 3 * MUL, MUL), k_vv.reshape(*Ssp, 3 * MUL, 3 * MUL)],
        axis=-1,
    )
    kernel = np.concatenate([top, bot], axis=-2)  # [5,5,5,in,out]
    return np.ascontiguousarray(np.transpose(kernel, (4, 3, 0, 1, 2)))


def _fold_self_connection(K, w_sc0, w_sc1):
    """Add the cropped e3nn Linear self-connection into the center tap."""
    inv = np.float32(1.0 / MUL ** 0.5)
    sc = np.zeros((64, 64), np.float32)
    sc[:MUL, :MUL] = w_sc0.T * inv  # sc[out w, in u] = w_sc0[u, w]
    for wo in range(MUL):
        for u in range(MUL):
            for k in range(3):
                sc[MUL + 3 * wo + k, MUL + 3 * u + k] += w_sc1[u, wo] * inv
    K = K.copy()
    K[:, :, CROP, CROP, CROP] += sc
    return K


def _pack_weights(K):
    """[64,64,5,5,5] fp32 -> DoubleRow lhsT tiles [128, NG, 2, 128] fp8.

    Row-half r of k-tile t holds the blocks for plane slot+r; column half m
    (out plane z+m) holds tap dz = plane - m from Khi or Klo per the group's
    class assignment."""
    Ks = K * SK
    Khi = Ks.astype(E4).astype(np.float32)
    Klo = (Ks - Khi).astype(E4).astype(np.float32)
    wk = np.zeros((128, NG, 2, 128), np.float32)
    for g, (dy, dx, kind, sA, sB, cA, cB) in enumerate(GROUPS):
        for t, (slot, cls) in enumerate(((sA, cA), (sB, cB))):
            if cls is None:
                continue
            Kc = Khi if cls == 0 else Klo
            for r in range(2):
                p = slot + r
                for m in range(2):
                    dz = p - m
                    if 0 <= dz < 5:
                        wk[64 * r : 64 * r + 64, g, t, 64 * m : 64 * m + 64] = Kc[
                            :, :, dz, dy, dx
                        ].T
    return np.ascontiguousarray(wk.astype(E4))


def _pack_x(x):
    """x [1,64,56,56,56] -> per-core fp8 hi/lo slabs:
    (hi [64,10,56,56], lo [64,10,56,56], hi2 [64,6,17,56], lo2 [...])."""
    xs = x[0] * SX
    xhi = xs.astype(E4)
    xlo = (xs - xhi.astype(np.float32)).astype(E4)
    slabs = []
    for c in range(N_CORES):
        z0, p2, y2 = _core_assign(c)
        slabs.append((
            np.ascontiguousarray(xhi[:, z0 : z0 + D_DRAM]),
            np.ascontiguousarray(xlo[:, z0 : z0 + D_DRAM]),
            np.ascontiguousarray(xhi[:, p2 : p2 + P2_DRAM, y2 : y2 + P2_IN_LINES]),
            np.ascontiguousarray(xlo[:, p2 : p2 + P2_DRAM, y2 : y2 + P2_IN_LINES]),
        ))
    return slabs


# ------------------------------------------------------- device program
def build_nc(n_pairs=N_PAIRS, partial=True, repeat=1):
    fp8 = mybir.dt.float8e4
    fp32 = mybir.dt.float32
    nc = bacc.Bacc("TRN2", target_bir_lowering=False, debug=False,
                   num_devices=N_CORES)
    xh_d = nc.dram_tensor("xh", [64, D_DRAM, S, S], fp8, kind="ExternalInput").ap()
    xl_d = nc.dram_tensor("xl", [64, D_DRAM, S, S], fp8, kind="ExternalInput").ap()
    xh2_d = nc.dram_tensor("xh2", [64, P2_DRAM, P2_IN_LINES, S], fp8,
                           kind="ExternalInput").ap()
    xl2_d = nc.dram_tensor("xl2", [64, P2_DRAM, P2_IN_LINES, S], fp8,
                           kind="ExternalInput").ap()
    w_d = nc.dram_tensor("w", [128, NG, 2, 128], fp8, kind="ExternalInput").ap()
    # outputs are plane-major so one DMA can write both planes of a pair:
    # SBUF partitions (z c) = plane-half * 64 + channel
    o_d = nc.dram_tensor("out", [2 * n_pairs, 64, SO, SO], fp32,
                         kind="ExternalOutput").ap()
    o2_d = nc.dram_tensor("out2", [2, 64, P2_LINES, SO], fp32,
                          kind="ExternalOutput").ap()

    DR = mybir.MatmulPerfMode.DoubleRow

    with tile.TileContext(nc) as tc:
        with (
            tc.tile_pool(name="const", bufs=1) as cpool,
            tc.tile_pool(name="outp", bufs=3) as opool,
            tc.tile_pool(name="psum", bufs=8, space="PSUM") as ppool,
        ):
            xh = cpool.tile([128, D_SB, S, S], fp8)
            xl = cpool.tile([128, D_SB, S, S], fp8)
            xh2 = cpool.tile([128, P2_SB, P2_IN_LINES, S], fp8)
            xl2 = cpool.tile([128, P2_SB, P2_IN_LINES, S], fp8)
            wt = cpool.tile([128, NG, 2, 128], fp8)
            # DMA order = first-use order, split across BOTH HWDGE queues
            # (SP = nc.sync, Activation = nc.scalar).  Lead DMAs cover only
            # what the first chunk's phase-0 groups touch (partial slab
            # slots 0..3, lines 0..10) so the first matmul fires early.
            # Dual z-shifted SBUF copies: partitions 0..63 plane j <- plane
            # j, partitions 64..127 plane j <- plane j+1.  Weight slices are
            # interleaved 2:1 ahead of the x planes: fp8 matmuls drain a
            # weight slice (3 groups) every ~260ns, faster than a 1:1
            # interleave can supply them.
            # head: weight slice 0 + partial-slab leads (slots 0..3, lines
            # 0..10 — exactly what the first chunk's phase-0 groups read)
            nc.sync.dma_start(xh2[:64], xh2_d[:, :P2_SB])
            nc.sync.dma_start(xh2[64:], xh2_d[:, 1 : P2_SB + 1])
            nc.sync.dma_start(xl2[:64], xl2_d[:, :P2_SB])
            nc.sync.dma_start(xl2[64:], xl2_d[:, 1 : P2_SB + 1])
            wops = [
                lambda i=i: nc.sync.dma_start(wt[:, 3 * i : 3 * (i + 1)],
                                              w_d[:, 3 * i : 3 * (i + 1)])
                for i in range(NG // 3)
            ]
            xops = []
            for j in range(D_SB):
                xops.append(lambda j=j: nc.sync.dma_start(xh[:64, j], xh_d[:, j]))
                xops.append(
                    lambda j=j: nc.sync.dma_start(xh[64:, j], xh_d[:, j + 1])
                )
            for j in range(D_SB):
                xops.append(lambda j=j: nc.sync.dma_start(xl[:64, j], xl_d[:, j]))
                xops.append(
                    lambda j=j: nc.sync.dma_start(xl[64:, j], xl_d[:, j + 1])
                )
            while wops or xops:
                if wops:
                    wops.pop(0)()
                if xops:
                    xops.pop(0)()

            def do_chunk(hi, lo, z, ys, L, dst, zo):
                ps = ppool.tile([128, 9, SO], fp32)
                for g, (dy, dx, kind, sA, sB, cA, cB) in enumerate(GROUPS):
                    src = hi if kind == 0 else lo
                    rhs = src[:, z + sA : z + sB + 1 : (sB - sA),
                              ys + dy : ys + dy + L, dx : dx + SO]
                    nc.tensor.matmul(ps[:, :L, :], wt[:, g], rhs,
                                     start=(g == 0), stop=(g == NG - 1),
                                     perf_mode=DR)
                ot = opool.tile([128, 9, SO], fp32)
                nc.vector.tensor_copy(ot[:, :L], ps[:, :L])
                dst2 = dst[zo : zo + 2, :, ys : ys + L, :].rearrange(
                    "z c l w -> (z c) l w"
                )
                nc.sync.dma_start(dst2, ot[:, :L])

            for _ in range(repeat):
                # partial first: its input lands quickly, hiding the main
                # slab's DMA behind the partial block's compute
                if partial:
                    for ys, L in CHUNKS2:
                        do_chunk(xh2, xl2, 0, ys, L, o2_d, 0)
                for p in range(n_pairs):
                    chunks = CHUNKS_LAST if p == n_pairs - 1 else CHUNKS
                    for ys, L in chunks:
                        do_chunk(xh, xl, 2 * p, ys, L, o_d, 2 * p)
    nc.compile()
    return nc


# ------------------------------------------------------------ entry point
LAST_RESULTS = None
LAST_NC = None
LAST_INMAPS = None


def kernel(x, weight, w_sc0, w_sc1):
    global LAST_RESULTS, LAST_NC, LAST_INMAPS
    x = np.asarray(x, dtype=np.float32)
    K = _fold_self_connection(
        _make_kernel(np.asarray(weight, dtype=np.float32)),
        np.asarray(w_sc0, dtype=np.float32),
        np.asarray(w_sc1, dtype=np.float32),
    )
    wk = _pack_weights(K)
    slabs = _pack_x(x)

    repeat = int(os.environ.get("KERNEL_REPEAT", "1"))
    nc = build_nc(repeat=repeat)
    in_maps = [
        {"xh": slabs[c][0], "xl": slabs[c][1], "xh2": slabs[c][2],
         "xl2": slabs[c][3], "w": wk}
        for c in range(N_CORES)
    ]
    res = run_bass_kernel_spmd(nc, in_maps, core_ids=list(range(N_CORES)))
    LAST_RESULTS, LAST_NC, LAST_INMAPS = res, nc, in_maps

    full = np.zeros((1, 64, SO, SO, SO), np.float32)
    for c in range(N_CORES):
        z0, p2, y2 = _core_assign(c)
        # device outputs are plane-major [z, c, l, w]
        full[0, :, z0 : z0 + D_OUT] = res.results[c]["out"].transpose(1, 0, 2, 3)
        full[0, :, p2 : p2 + 2, y2 : y2 + P2_LINES, :] = res.results[c][
            "out2"
        ].transpose(1, 0, 2, 3)
    return full * OUT_SCALE
